# revision 1
# baseline (speedup 1.0000x reference)
"""DiffusionTransformer (AF3-style) Trainium2 kernel, 8-core SPMD.

Sharding: sequence-parallel over rows (queries). Each core owns R=128 rows of
a / z / beta. Per block, k/v are computed on local rows and AllGathered
(bf16, ~384KB per rank). zbeta (pair bias LN(z)@wpb + beta, all 4 blocks) is
precomputed once into per-core DRAM in [bh, i, j] layout.
"""
import numpy as np
import ml_dtypes
from contextlib import ExitStack

import concourse.bass as bass
import concourse.mybir as mybir
import concourse.tile as tile
from concourse import bacc
from concourse.bass_utils import run_bass_kernel_spmd
from concourse.masks import make_identity

NB, H, S, CA, CS, CZ = 4, 16, 1024, 768, 384, 128
D = CA // H            # 48
NCORE = 8
R = S // NCORE         # 128 rows per core
NHID = 2 * CA          # 1536
EPS = 1e-5
BF = mybir.dt.bfloat16
F32 = mybir.dt.float32
AX = mybir.AxisListType
OP = mybir.AluOpType
ACTF = mybir.ActivationFunctionType
JT = S // 128          # 8 j-tiles
KT_A = CA // 128       # 6 k-tiles over c_a
KT_S = CS // 128       # 3 k-tiles over c_s
KT_H = NHID // 128     # 12 k-tiles over n_hidden
KV_K = D * H * R       # 98304 flat elems of kT part
KV_TOT = KV_K + R * CA  # + v part


def _headsplit_ranges():
    """Split [0,768) hd-range at both 128-tile and 48-head boundaries.

    Returns list of (kt, p0, p1, h, d0) with kt*128+p0 == h*48+d0.
    """
    out = []
    bounds = sorted(set([x * 128 for x in range(KT_A + 1)] +
                        [h * D for h in range(H + 1)]))
    for lo, hi in zip(bounds[:-1], bounds[1:]):
        kt, p0 = lo // 128, lo % 128
        h, d0 = lo // D, lo % D
        out.append((kt, p0, hi - lo, h, d0))
    return out


HS = _headsplit_ranges()


def build_program(bias_pb):
    nc = bacc.Bacc("TRN2", target_bir_lowering=False, debug=False,
                   num_devices=NCORE)

    # ---------------- I/O ----------------
    a_in = nc.dram_tensor("a_in", [R, CA], F32, kind="ExternalInput")
    s_in = nc.dram_tensor("s_in", [R, CS], F32, kind="ExternalInput")
    z_in = nc.dram_tensor("z_in", [R, S, CZ], BF, kind="ExternalInput")
    betaT_in = nc.dram_tensor("betaT_in", [H, R, S], BF, kind="ExternalInput")
    wz_in = nc.dram_tensor("wz_in", [CZ, NB * H], BF, kind="ExternalInput")
    sel_in = nc.dram_tensor("sel_in", [H, NB * H], BF, kind="ExternalInput")
    wsn_in = nc.dram_tensor("wsn_in", [CS, NB, 4 * CA], BF, kind="ExternalInput")
    wsr_in = nc.dram_tensor("wsr_in", [CS + 1, NB, 2 * CA], BF, kind="ExternalInput")
    wkv_in = nc.dram_tensor("wkv_in", [CA, NB, 2 * CA], BF, kind="ExternalInput")
    wqg_in = nc.dram_tensor("wqg_in", [CA, NB, 2 * CA], BF, kind="ExternalInput")
    bq_in = nc.dram_tensor("bq_in", [NB, CA], F32, kind="ExternalInput")
    wsw_in = nc.dram_tensor("wsw_in", [CA, NB, NHID], BF, kind="ExternalInput")
    wg2_in = nc.dram_tensor("wg2_in", [CA, NB, NHID], BF, kind="ExternalInput")
    wo_in = nc.dram_tensor("wo_in", [CA, NB, CA], BF, kind="ExternalInput")
    wout_in = nc.dram_tensor("wout_in", [NHID, NB, CA], BF, kind="ExternalInput")
    a_out = nc.dram_tensor("a_out", [R, CA], F32, kind="ExternalOutput")

    with tile.TileContext(nc) as tc, ExitStack() as ctx:
        const = ctx.enter_context(tc.tile_pool(name="const", bufs=1))
        ident = const.tile([128, 128], BF)
        make_identity(nc, ident)
        wz_sb = const.tile([CZ, NB * H], BF)
        nc.sync.dma_start(out=wz_sb, in_=wz_in[:])
        sel_sb = const.tile([H, NB * H], BF)
        nc.sync.dma_start(out=sel_sb, in_=sel_in[:])
        biases = const.tile([128, 1 + NB * H], F32)
        nc.vector.memset(biases[:, 0:1], EPS)
        for _bh in range(NB * H):
            nc.vector.memset(biases[:, 1 + _bh:2 + _bh], float(bias_pb[_bh]))

        pers = ctx.enter_context(tc.tile_pool(name="pers", bufs=1))
        a_sb = pers.tile([R, CA], F32)
        nc.sync.dma_start(out=a_sb, in_=a_in[:])

        # internal DRAM scratch
        dram = ctx.enter_context(tc.tile_pool(name="dram", bufs=1, space="DRAM"))
        zbeta_dr = dram.tile([NB * H, R, S], BF)

        # =========== s preprocessing (once) ===========
        with tc.tile_pool(name="sprep", bufs=1) as sp:
            s_sb = sp.tile([R, CS], F32)
            nc.sync.dma_start(out=s_sb, in_=s_in[:])
            stats = sp.tile([R, 1, 6], F32)
            mv = sp.tile([R, 2], F32)
            nc.vector.bn_stats(out=stats[:, 0, :], in_=s_sb[:])
            nc.vector.bn_aggr(out=mv, in_=stats)
            rstd = sp.tile([R, 1], F32)
            nc.scalar.activation(out=rstd, in_=mv[:, 1:2], func=ACTF.Sqrt,
                                 bias=biases[:, 0:1], scale=1.0)
            nc.vector.reciprocal(out=rstd, in_=rstd)
            s_n = sp.tile([R, CS], BF)
            nc.vector.tensor_scalar(s_n[:], s_sb[:], mv[:, 0:1], rstd[:, 0:1],
                                    OP.subtract, OP.mult)
            s_b16 = sp.tile([R, CS], BF)
            nc.vector.tensor_copy(s_b16[:], s_sb[:])

            # transposed copies (persistent for all blocks)
            s_nT = pers.tile([128, KT_S, 128], BF)
            sT = pers.tile([128, KT_S, 128], BF)
            ones_row = pers.tile([1, 128], BF)
            nc.vector.memset(ones_row, 1.0)
            with tc.tile_pool(name="tp_ps", bufs=2, space="PSUM") as tps:
                for kt in range(KT_S):
                    pt = tps.tile([128, 128], BF, tag="t")
                    nc.tensor.transpose(pt[:], s_n[:, bass.ts(kt, 128)], ident[:])
                    nc.scalar.copy(out=s_nT[:, kt, :], in_=pt[:])
                    pt2 = tps.tile([128, 128], BF, tag="t")
                    nc.tensor.transpose(pt2[:], s_b16[:, bass.ts(kt, 128)], ident[:])
                    nc.scalar.copy(out=sT[:, kt, :], in_=pt2[:])

            # per-block s-derived tensors: sg/sb for attn+tr, gates attn/tr
            sgsb = pers.tile([R, NB, 4 * CA], BF)   # wg_a|wb_a|wg_t|wb_t
            gts = pers.tile([R, NB, 2 * CA], BF)    # gate_attn|gate_tr
            with tc.tile_pool(name="sw", bufs=2) as swp, \
                 tc.tile_pool(name="sps", bufs=3, space="PSUM") as sps:
                for b in range(NB):
                    wsn_sb = swp.tile([128, KT_S, 4 * CA], BF, tag="wsn")
                    nc.sync.dma_start(
                        out=wsn_sb,
                        in_=wsn_in[:].rearrange("(kt p) b m -> p kt b m", p=128)[:, :, b, :])
                    for m in range(6):  # 3072 / 512
                        ps = sps.tile([R, 512], F32, tag="ps")
                        for kt in range(KT_S):
                            nc.tensor.matmul(ps[:], s_nT[:, kt, :],
                                             wsn_sb[:, kt, bass.ts(m, 512)],
                                             start=(kt == 0), stop=(kt == KT_S - 1))
                        # cols [m*512,(m+1)*512) of [wg_a(768)|wb_a|wg_t|wb_t]
                        for lo_, hi_ in [(m * 512, m * 512 + 256), (m * 512 + 256, (m + 1) * 512)]:
                            mat = lo_ // CA  # 0..3
                            f = ACTF.Sigmoid if mat in (0, 2) else ACTF.Copy
                            nc.scalar.activation(
                                out=sgsb[:, b, lo_:hi_], in_=ps[:, lo_ - m * 512:hi_ - m * 512],
                                func=f, bias=0.0 if f == ACTF.Copy else 0.0, scale=1.0)
                    wsr_sb = swp.tile([128, KT_S, 2 * CA], BF, tag="wsr")
                    nc.sync.dma_start(
                        out=wsr_sb,
                        in_=wsr_in[:CS].rearrange("(kt p) b m -> p kt b m", p=128)[:, :, b, :])
                    wsr_last = swp.tile([1, 2 * CA], BF, tag="wsrl")
                    nc.sync.dma_start(out=wsr_last, in_=wsr_in[CS:CS + 1, b, :])
                    for m in range(3):  # 1536 / 512
                        ps = sps.tile([R, 512], F32, tag="ps")
                        for kt in range(KT_S):
                            nc.tensor.matmul(ps[:], sT[:, kt, :],
                                             wsr_sb[:, kt, bass.ts(m, 512)],
                                             start=(kt == 0), stop=False)
                        nc.tensor.matmul(ps[:], ones_row[:],
                                         wsr_last[:, bass.ts(m, 512)],
                                         start=False, stop=True)
                        nc.scalar.activation(out=gts[:, b, bass.ts(m, 512)],
                                             in_=ps[:], func=ACTF.Sigmoid,
                                             bias=0.0, scale=1.0)

        # =========== z preprocessing (once) ===========
        with tc.tile_pool(name="zslab", bufs=4) as zsl, \
             tc.tile_pool(name="zsm", bufs=4) as zsm, \
             tc.tile_pool(name="ztp", bufs=3, space="PSUM") as ztp, \
             tc.tile_pool(name="zbp", bufs=2, space="PSUM") as zbp:
            z_r = z_in[:].rearrange("i (jt jp) c -> i jp jt c", jp=128)
            for i in range(R):
                zt = zsl.tile([128, JT, CZ], BF, tag="z")
                for _q in range(4):
                    nc.sync.dma_start(out=zt[:, 2 * _q:2 * _q + 2, :],
                                      in_=z_r[i][:, 2 * _q:2 * _q + 2, :])
                bsel = zsl.tile([H, S], BF, tag="bsel")
                nc.sync.dma_start(out=bsel, in_=betaT_in[:, i, :])
                st8 = zsm.tile([128, JT, 6], F32, tag="st")
                for jt in range(JT):
                    nc.vector.bn_stats(out=st8[:, jt, :], in_=zt[:, jt, :])
                # pooled even/odd moments, vectorized over all 8 j-tiles:
                # mean = (m_e+m_o)/2; var = (64v_e+64v_o)/128 + ((m_e-m_o)/2)^2
                mrow = zsm.tile([128, JT], F32, tag="mrow")
                nc.vector.tensor_tensor(mrow[:], st8[:, :, 1], st8[:, :, 4], OP.add)
                nc.vector.tensor_scalar(mrow[:], mrow[:], 0.5, None, OP.mult)
                dm = zsm.tile([128, JT], F32, tag="dm")
                nc.vector.tensor_tensor(dm[:], st8[:, :, 1], st8[:, :, 4],
                                        OP.subtract)
                nc.vector.tensor_tensor(dm[:], dm[:], dm[:], OP.mult)
                nc.vector.tensor_scalar(dm[:], dm[:], 0.25, None, OP.mult)
                var = zsm.tile([128, JT], F32, tag="var")
                nc.vector.tensor_tensor(var[:], st8[:, :, 2], st8[:, :, 5], OP.add)
                nc.vector.tensor_scalar(var[:], var[:], 1.0 / CZ, None, OP.mult)
                nc.vector.tensor_tensor(var[:], var[:], dm[:], OP.add)
                rst = zsm.tile([128, JT], F32, tag="rst")
                nc.scalar.activation(out=rst, in_=var[:], func=ACTF.Sqrt,
                                     bias=biases[:, 0:1], scale=1.0)
                nc.vector.reciprocal(out=rst, in_=rst)
                zh = zsm.tile([128, JT, CZ], BF, tag="zh")
                for jt in range(JT):
                    nc.vector.tensor_scalar(zh[:, jt, :], zt[:, jt, :],
                                            mrow[:, jt:jt + 1], rst[:, jt:jt + 1],
                                            OP.subtract, OP.mult)
                zhT = zsm.tile([128, JT, 128], BF, tag="zhT")
                for g in range(2):  # transpose 8 tiles, copy in 2 batches
                    pt = ztp.tile([128, 4, 128], BF, tag="t")
                    for q in range(4):
                        jt = g * 4 + q
                        nc.tensor.transpose(pt[:, q, :], zh[:, jt, :], ident[:])
                    nc.scalar.copy(out=zhT[:, g * 4:(g + 1) * 4, :], in_=pt[:])
                zb = zbp.tile([NB * H, S], F32, tag="zb")
                for jc in range(2):
                    nc.tensor.matmul(zb[:, bass.ts(jc, 512)], wz_sb[:],
                                     zhT[:].rearrange("p jt j -> p (jt j)")[:, bass.ts(jc, 512)],
                                     start=True, stop=False)
                    nc.tensor.matmul(zb[:, bass.ts(jc, 512)], sel_sb[:],
                                     bsel[:, bass.ts(jc, 512)],
                                     start=False, stop=True)
                zbs = zsm.tile([NB * H, S], BF, tag="zbs")
                nc.scalar.copy(out=zbs, in_=zb[:])
                nc.sync.dma_start(out=zbeta_dr[:, i, :], in_=zbs)

        # =========== block loop ===========
        wpool = ctx.enter_context(tc.tile_pool(name="wpool", bufs=2))
        wop = ctx.enter_context(tc.tile_pool(name="wop", bufs=1))
        blk = ctx.enter_context(tc.tile_pool(name="blk", bufs=1))
        kvg = ctx.enter_context(tc.tile_pool(name="kvg", bufs=1))
        att = ctx.enter_context(tc.tile_pool(name="att", bufs=3))
        dramc = ctx.enter_context(tc.tile_pool(name="dramc", bufs=2, space="DRAM"))

        for b in range(NB):
            # ---- ada_ln(a) shared stats ----
            with tc.tile_pool(name="lnp", bufs=1) as lnp, \
                 tc.tile_pool(name="ps_ln", bufs=2, space="PSUM") as pln:
                st3 = lnp.tile([R, 3, 6], F32)
                for g_ in range(3):
                    nc.vector.bn_stats(out=st3[:, g_, :],
                                       in_=a_sb[:, bass.ts(g_, 256)])
                mv = lnp.tile([R, 2], F32)
                nc.vector.bn_aggr(out=mv, in_=st3)
                rstd = lnp.tile([R, 1], F32)
                nc.scalar.activation(out=rstd, in_=mv[:, 1:2], func=ACTF.Sqrt,
                                     bias=biases[:, 0:1], scale=1.0)
                nc.vector.reciprocal(out=rstd, in_=rstd)
                xhat = lnp.tile([R, CA], BF)
                nc.vector.tensor_scalar(xhat[:], a_sb[:], mv[:, 0:1], rstd[:, 0:1],
                                        OP.subtract, OP.mult)
                ah = blk.tile([R, CA], BF, tag="ah")
                nc.vector.tensor_tensor(ah[:], xhat[:], sgsb[:, b, 0:CA], OP.mult)
                nc.vector.tensor_tensor(ah[:], ah[:], sgsb[:, b, CA:2 * CA], OP.add)
                th = blk.tile([R, CA], BF, tag="th")
                nc.vector.tensor_tensor(th[:], xhat[:], sgsb[:, b, 2 * CA:3 * CA], OP.mult)
                nc.vector.tensor_tensor(th[:], th[:], sgsb[:, b, 3 * CA:4 * CA], OP.add)
                ahT = blk.tile([128, KT_A, 128], BF, tag="ahT")
                for kt in range(KT_A):
                    pt = pln.tile([128, 128], BF, tag="t")
                    nc.tensor.transpose(pt[:], ah[:, bass.ts(kt, 128)], ident[:])
                    nc.scalar.copy(out=ahT[:, kt, :], in_=pt[:])
                thT = blk.tile([128, KT_A, 128], BF, tag="thT")
                for kt in range(KT_A):
                    pt = pln.tile([128, 128], BF, tag="t")
                    nc.tensor.transpose(pt[:], th[:, bass.ts(kt, 128)], ident[:])
                    nc.scalar.copy(out=thT[:, kt, :], in_=pt[:])

            # ---- kv local + gather ----
            kv_inb = dramc.tile([KV_TOT], BF, tag="kvin")
            kv_outb = dramc.tile([NCORE * KV_TOT], BF, tag="kvout",
                                 addr_space="Shared")
            with tc.tile_pool(name="ps_kv", bufs=3, space="PSUM") as pkv:
                wkv_sb = wpool.tile([128, KT_A, 2 * CA], BF, tag="w1536")
                nc.sync.dma_start(
                    out=wkv_sb,
                    in_=wkv_in[:].rearrange("(kt p) b m -> p kt b m", p=128)[:, :, b, :])
                kv_sb = kvg.tile([R, 2 * CA], BF, tag="kv")
                for m in range(3):  # 1536/512
                    ps = pkv.tile([R, 512], F32, tag="ps")
                    for kt in range(KT_A):
                        nc.tensor.matmul(ps[:], ahT[:, kt, :],
                                         wkv_sb[:, kt, bass.ts(m, 512)],
                                         start=(kt == 0), stop=(kt == KT_A - 1))
                    nc.scalar.copy(out=kv_sb[:, bass.ts(m, 512)], in_=ps[:])
                # kT head-tiles
                kT_loc = kvg.tile([D, H, 128], BF, tag="kT")
                for h in range(H):
                    pt = pkv.tile([128, 128], BF, tag="t")
                    nc.tensor.transpose(pt[:D, :], kv_sb[:, h * D:(h + 1) * D], ident[:])
                    nc.vector.tensor_copy(kT_loc[:, h, :], pt[:D, :])
                nc.sync.dma_start(out=kv_inb[0:KV_K].rearrange("(d x) -> d x", d=D),
                                  in_=kT_loc[:].rearrange("d h j -> d (h j)"))
                nc.sync.dma_start(out=kv_inb[KV_K:].rearrange("(j c) -> j c", j=R),
                                  in_=kv_sb[:, CA:])
            nc.gpsimd.collective_compute(
                "AllGather", OP.bypass,
                replica_groups=[list(range(NCORE))],
                ins=[kv_inb[:].opt()], outs=[kv_outb[:].opt()])

            # ---- qT, gT (transposed head layout) ----
            with tc.tile_pool(name="ps_qg", bufs=3, space="PSUM") as pqg:
                wqg_sb = wpool.tile([128, KT_A, 2 * CA], BF, tag="w1536")
                nc.sync.dma_start(
                    out=wqg_sb,
                    in_=wqg_in[:].rearrange("(kt p) b m -> p kt b m", p=128)[:, :, b, :])
                bq_sb = blk.tile([D, H], F32, tag="bq")
                nc.sync.dma_start(
                    out=bq_sb, in_=bq_in[b].rearrange("(h d) -> d h", d=D))
                qT = blk.tile([D, H, 128], BF, tag="qT")
                gT = blk.tile([D, H, 128], BF, tag="gT")
                for h in range(H):
                    ps = pqg.tile([D, 128], F32, tag="ps")
                    for kt in range(KT_A):
                        nc.tensor.matmul(ps[:], wqg_sb[:, kt, h * D:(h + 1) * D],
                                         ahT[:, kt, :],
                                         start=(kt == 0), stop=(kt == KT_A - 1))
                    nc.scalar.activation(out=qT[:, h, :], in_=ps[:],
                                         func=ACTF.Identity,
                                         bias=bq_sb[:, h:h + 1], scale=1.0)
                    ps2 = pqg.tile([D, 128], F32, tag="ps")
                    for kt in range(KT_A):
                        nc.tensor.matmul(ps2[:], wqg_sb[:, kt, CA + h * D:CA + (h + 1) * D],
                                         ahT[:, kt, :],
                                         start=(kt == 0), stop=(kt == KT_A - 1))
                    nc.scalar.activation(out=gT[:, h, :], in_=ps2[:],
                                         func=ACTF.Sigmoid, bias=0.0, scale=1.0)

            # ---- transition: hiddenT = silu(th@wsw)^T * (th@wg2)^T ----
            hidT = blk.tile([128, KT_H, 128], BF, tag="hidT")
            with tc.tile_pool(name="ps_h", bufs=3, space="PSUM") as psh:
                wsw_sb = wpool.tile([128, KT_A, NHID], BF, tag="w1536")
                nc.sync.dma_start(
                    out=wsw_sb,
                    in_=wsw_in[:].rearrange("(kt p) b m -> p kt b m", p=128)[:, :, b, :])
                swT = blk.tile([128, KT_H, 128], BF, tag="swT")
                for mt in range(KT_H):
                    ps = psh.tile([128, 128], F32, tag="ps")
                    for kt in range(KT_A):
                        nc.tensor.matmul(ps[:], wsw_sb[:, kt, bass.ts(mt, 128)],
                                         thT[:, kt, :],
                                         start=(kt == 0), stop=(kt == KT_A - 1))
                    nc.scalar.activation(out=swT[:, mt, :], in_=ps[:],
                                         func=ACTF.Silu, bias=0.0, scale=1.0)
                wg2_sb = wpool.tile([128, KT_A, NHID], BF, tag="w1536")
                nc.sync.dma_start(
                    out=wg2_sb,
                    in_=wg2_in[:].rearrange("(kt p) b m -> p kt b m", p=128)[:, :, b, :])
                for mt in range(KT_H):
                    ps = psh.tile([128, 128], F32, tag="ps")
                    for kt in range(KT_A):
                        nc.tensor.matmul(ps[:], wg2_sb[:, kt, bass.ts(mt, 128)],
                                         thT[:, kt, :],
                                         start=(kt == 0), stop=(kt == KT_A - 1))
                    g2 = att.tile([128, 128], BF, tag="g2")
                    nc.scalar.copy(out=g2, in_=ps[:])
                    nc.vector.tensor_tensor(hidT[:, mt, :], swT[:, mt, :], g2[:],
                                            OP.mult)

            # ---- attention ----
            kv_outr = kv_outb[:].rearrange("(r x) -> r x", r=NCORE)
            kT_src = kv_outr[:, 0:KV_K].rearrange(
                "r (d h j) -> d h r j", d=D, h=H)
            v_full = kvg.tile([128, NCORE, CA], BF, tag="vf")
            v_src = kv_outb[:].rearrange("(r x) -> r x", r=NCORE)[:, KV_K:] \
                .rearrange("r (j c) -> j r c", j=R)
            for _q in range(4):
                nc.sync.dma_start(out=v_full[:, 2 * _q:2 * _q + 2, :],
                                  in_=v_src[:, 2 * _q:2 * _q + 2, :])
            go_T = blk.tile([D, H, 128], BF, tag="goT")
            sums = blk.tile([R, H], F32, tag="sums")
            with tc.tile_pool(name="ps_s", bufs=2, space="PSUM") as pss, \
                 tc.tile_pool(name="ps_t", bufs=2, space="PSUM") as pst, \
                 tc.tile_pool(name="ps_o", bufs=2, space="PSUM") as pso:
                for h in range(H):
                    kT_h = att.tile([D, NCORE, 128], BF, tag="kTh")
                    nc.sync.dma_start(out=kT_h[:, 0:4, :], in_=kT_src[:, h, 0:4, :])
                    nc.sync.dma_start(out=kT_h[:, 4:8, :], in_=kT_src[:, h, 4:8, :])
                    ps_s = pss.tile([R, S], F32, tag="s")
                    for jc in range(2):
                        nc.tensor.matmul(ps_s[:, bass.ts(jc, 512)], qT[:, h, :],
                                         kT_h[:, jc * 4:(jc + 1) * 4, :],
                                         start=True, stop=True)
                    zb_t = att.tile([R, S], BF, tag="zbt")
                    nc.sync.dma_start(out=zb_t[:, 0:512],
                                      in_=zbeta_dr[b * H + h, :, 0:512])
                    nc.sync.dma_start(out=zb_t[:, 512:1024],
                                      in_=zbeta_dr[b * H + h, :, 512:1024])
                    nc.vector.tensor_tensor(ps_s[:], ps_s[:], zb_t[:], OP.add)
                    attn = att.tile([R, S], BF, tag="attn")
                    bh_ = 1 + b * H + h
                    nc.scalar.activation(out=attn, in_=ps_s[:], func=ACTF.Exp,
                                         bias=biases[:, bh_:bh_ + 1], scale=1.0,
                                         accum_out=sums[:, h:h + 1])
                    rec = att.tile([R, 1], F32, tag="rec")
                    nc.vector.reciprocal(out=rec, in_=sums[:, h:h + 1])
                    nc.vector.tensor_scalar(attn[:], attn[:], rec[:, 0:1], None,
                                            OP.mult)
                    attnT = att.tile([128, JT, 128], BF, tag="attnT")
                    for g in range(2):
                        pt = pst.tile([128, 4, 128], BF, tag="t")
                        for q in range(4):
                            jt = g * 4 + q
                            nc.tensor.transpose(pt[:, q, :],
                                                attn[:, bass.ts(jt, 128)], ident[:])
                        nc.vector.tensor_copy(attnT[:, g * 4:(g + 1) * 4, :], pt[:])
                    ps_o = pso.tile([128, 128], F32, tag="o")
                    for jt in range(JT):
                        nc.tensor.matmul(ps_o[:D, :], v_full[:, jt, h * D:(h + 1) * D],
                                         attnT[:, jt, :],
                                         start=(jt == 0), stop=(jt == JT - 1))
                    nc.vector.tensor_tensor(go_T[:, h, :], ps_o[:D, :],
                                            gT[:, h, :], OP.mult)

            # ---- att_out = (g*o) @ wo ; b_attn = gate_attn * att_out ----
            b_attn = blk.tile([R, CA], F32, tag="batt")
            with tc.tile_pool(name="ps_wo", bufs=2, space="PSUM") as pwo:
                wo_sb = wop.tile([D, H, CA], BF, tag="wo")
                nc.sync.dma_start(
                    out=wo_sb,
                    in_=wo_in[:].rearrange("(h d) b m -> d h b m", d=D)[:, :, b, :])
                for m in range(2):
                    n0, n1 = (0, 512) if m == 0 else (512, 768)
                    ps = pwo.tile([R, 512], F32, tag="ps")
                    for h in range(H):
                        nc.tensor.matmul(ps[:, 0:n1 - n0], go_T[:, h, :],
                                         wo_sb[:, h, n0:n1],
                                         start=(h == 0), stop=(h == H - 1))
                    nc.vector.tensor_tensor(b_attn[:, n0:n1], ps[:, 0:n1 - n0],
                                            gts[:, b, n0:n1], OP.mult)

            # ---- tr = gate_tr * (hidden @ w_out); a = b_attn + tr ----
            with tc.tile_pool(name="ps_tr", bufs=2, space="PSUM") as ptr:
                wout_sb = wop.tile([128, KT_H, CA], BF, tag="wout")
                nc.sync.dma_start(
                    out=wout_sb,
                    in_=wout_in[:].rearrange("(kt p) b m -> p kt b m", p=128)[:, :, b, :])
                for m in range(2):
                    n0, n1 = (0, 512) if m == 0 else (512, 768)
                    ps = ptr.tile([R, 512], F32, tag="ps")
                    for kt in range(KT_H):
                        nc.tensor.matmul(ps[:, 0:n1 - n0], hidT[:, kt, :],
                                         wout_sb[:, kt, n0:n1],
                                         start=(kt == 0), stop=(kt == KT_H - 1))
                    tr = att.tile([R, 512], F32, tag="tr")
                    nc.vector.tensor_tensor(tr[:, 0:n1 - n0], ps[:, 0:n1 - n0],
                                            gts[:, b, CA + n0:CA + n1], OP.mult)
                    nc.vector.tensor_tensor(a_sb[:, n0:n1], b_attn[:, n0:n1],
                                            tr[:, 0:n1 - n0], OP.add)

        nc.sync.dma_start(out=a_out[:], in_=a_sb[:])

    nc.finalize()
    return nc


def _prep_inputs(a, s, z, beta, ln_s_w_attn, wg_attn, wb_attn, wq, bq, wk, wv,
                 ln_z_w, ln_z_b, wpb, wgate, wo, wsg_attn, bsg_attn,
                 ln_s_w_tr, wg_tr, wb_tr, w_swish, w_gate2, wsg_tr, bsg_tr, w_out):
    bf = ml_dtypes.bfloat16
    f32 = np.float32
    scale = 1.0 / np.sqrt(np.float32(D))

    # folded weights (shared across cores)
    wz = np.concatenate([ln_z_w[i][:, None] * wpb[i] for i in range(NB)],
                        axis=1).astype(bf)                       # [CZ, NB*H]
    bias_pb = np.concatenate([ln_z_b[i] @ wpb[i] for i in range(NB)])  # [NB*H]
    sel = np.tile(np.eye(H, dtype=np.float32), (1, NB)).astype(bf)  # [H, NB*H]
    wsn = np.stack([np.concatenate(
        [ln_s_w_attn[i][:, None] * wg_attn[i], ln_s_w_attn[i][:, None] * wb_attn[i],
         ln_s_w_tr[i][:, None] * wg_tr[i], ln_s_w_tr[i][:, None] * wb_tr[i]],
        axis=1) for i in range(NB)], axis=1).astype(bf)          # [CS, NB, 4CA]
    wsr = np.stack([np.concatenate(
        [np.concatenate([wsg_attn[i], bsg_attn[i][None, :]], 0),
         np.concatenate([wsg_tr[i], bsg_tr[i][None, :]], 0)], axis=1)
        for i in range(NB)], axis=1).astype(bf)                  # [CS+1, NB, 2CA]
    wkv = np.stack([np.concatenate([wk[i], wv[i]], 1) for i in range(NB)],
                   axis=1).astype(bf)                            # [CA, NB, 2CA]
    wqg = np.stack([np.concatenate([wq[i] * scale, wgate[i]], 1)
                    for i in range(NB)], axis=1).astype(bf)
    bqe = (bq * scale).astype(f32)                               # [NB, CA]
    wsw = np.stack([w_swish[i] for i in range(NB)], axis=1).astype(bf)
    wg2 = np.stack([w_gate2[i] for i in range(NB)], axis=1).astype(bf)
    wob = np.stack([wo[i] for i in range(NB)], axis=1).astype(bf)
    wout = np.stack([w_out[i] for i in range(NB)], axis=1).astype(bf)

    shared = dict(wz_in=np.ascontiguousarray(wz),
                  sel_in=np.ascontiguousarray(sel),
                  wsn_in=np.ascontiguousarray(wsn),
                  wsr_in=np.ascontiguousarray(wsr),
                  wkv_in=np.ascontiguousarray(wkv),
                  wqg_in=np.ascontiguousarray(wqg),
                  bq_in=np.ascontiguousarray(bqe),
                  wsw_in=np.ascontiguousarray(wsw),
                  wg2_in=np.ascontiguousarray(wg2),
                  wo_in=np.ascontiguousarray(wob),
                  wout_in=np.ascontiguousarray(wout))

    a2 = a.reshape(S, CA).astype(f32)
    s2 = s.reshape(S, CS).astype(f32)
    z2 = z.reshape(S, S, CZ).astype(bf)
    betaT = np.ascontiguousarray(
        beta.reshape(S, S, H).transpose(2, 0, 1)).astype(bf)     # [H, S, S]

    in_maps = []
    for c in range(NCORE):
        rows = slice(c * R, (c + 1) * R)
        m = dict(shared)
        m["a_in"] = np.ascontiguousarray(a2[rows])
        m["s_in"] = np.ascontiguousarray(s2[rows])
        m["z_in"] = np.ascontiguousarray(z2[rows])
        m["betaT_in"] = np.ascontiguousarray(betaT[:, rows, :])
        in_maps.append(m)
    return in_maps, [float(x) for x in bias_pb]


_CACHE = {}


def kernel(**inputs):
    inputs = {k: np.asarray(v) for k, v in inputs.items()}
    in_maps, bias_pb = _prep_inputs(**inputs)
    key = tuple(bias_pb)
    if key not in _CACHE:
        _CACHE.clear()
        _CACHE[key] = build_program(bias_pb)
    nc = _CACHE[key]
    res = run_bass_kernel_spmd(nc, in_maps, core_ids=list(range(NCORE)),
                               trace=False)
    out = np.concatenate([res.results[c]["a_out"] for c in range(NCORE)], axis=0)
    return out.reshape(1, S, CA).astype(np.float32)


if __name__ == "__main__":
    import reference
    ins = {k: np.asarray(v) for k, v in reference.setup_inputs().items()}
    exp = np.asarray(reference.reference(**ins))
    act = kernel(**ins)
    err = np.abs(act - exp).max() / (np.abs(exp).max() + 1e-9)
    print("rel err:", err)



# revision 48
# speedup vs baseline: 1.1505x; 1.1505x over previous
"""DiffusionTransformer (AF3-style) Trainium2 kernel, 8-core SPMD, v2.

Sharding: sequence-parallel over rows (queries). Each core owns R=128 rows of
a / z / beta. Per block, k/v are computed on local rows (row-form matmuls),
packed, and AllGathered (bf16, ~384KB per rank); block-0's gather overlaps
the one-time z preprocessing. zbeta (pair bias LN(z)@wpb + beta, all 4
blocks) is computed in one pass over z into per-core DRAM in [bh, i, j]
layout and consumed as [i, j] strips per (block, head).
"""
import numpy as np
import ml_dtypes
from contextlib import ExitStack

import concourse.bass as bass
import concourse.mybir as mybir
import concourse.tile as tile
from concourse import bacc
from concourse.bass_utils import run_bass_kernel_spmd
from concourse.masks import make_identity

NB, H, S, CA, CS, CZ = 4, 16, 1024, 768, 384, 128
D = CA // H            # 48
NCORE = 8
R = S // NCORE         # 128 rows per core
NHID = 2 * CA          # 1536
EPS = 1e-5
BF = mybir.dt.bfloat16
F32 = mybir.dt.float32
AX = mybir.AxisListType
OP = mybir.AluOpType
ACTF = mybir.ActivationFunctionType
JT = S // 128          # 8 j-tiles
KT_A = CA // 128       # 6 k-tiles over c_a
KT_S = CS // 128       # 3 k-tiles over c_s
KT_H = NHID // 128     # 12 k-tiles over n_hidden
KV_TOT = R * 2 * CA    # flat bf16 elems gathered per rank (kT tiles + v rows)
KV_K = R * CA          # kT part


def _head_pieces():
    """Split each head's d-range [48h, 48h+48) at 128-tile boundaries.

    Returns per-head list of (kt, p0, plen, d0) with kt*128+p0 == 48h+d0.
    """
    out = []
    for h in range(H):
        lo, hi = h * D, (h + 1) * D
        pieces = []
        while lo < hi:
            kt, p0 = lo // 128, lo % 128
            plen = min(hi - lo, 128 - p0)
            pieces.append((kt, p0, plen, lo - h * D))
            lo += plen
        out.append(pieces)
    return out


HP = _head_pieces()


def build_program(bias_pb):
    nc = bacc.Bacc("TRN2", target_bir_lowering=False, debug=False,
                   num_devices=NCORE)

    # ---------------- I/O ----------------
    a_in = nc.dram_tensor("a_in", [R, CA], F32, kind="ExternalInput")
    s_in = nc.dram_tensor("s_in", [R, CS], F32, kind="ExternalInput")
    # z in [i][jp][jt*c] layout: partition lines are 2KB contiguous
    z_in = nc.dram_tensor("z_in", [R, 128, JT * CZ], BF, kind="ExternalInput")
    betaT_in = nc.dram_tensor("betaT_in", [H, R, S], BF, kind="ExternalInput")
    wz_in = nc.dram_tensor("wz_in", [CZ, NB * H], BF, kind="ExternalInput")
    sel_in = nc.dram_tensor("sel_in", [H, NB * H], BF, kind="ExternalInput")
    wsn_in = nc.dram_tensor("wsn_in", [CS, NB, 4 * CA], BF, kind="ExternalInput")
    wsr_in = nc.dram_tensor("wsr_in", [CS + 1, NB, 2 * CA], BF, kind="ExternalInput")
    wkv_in = nc.dram_tensor("wkv_in", [CA, NB, 2 * CA], BF, kind="ExternalInput")
    wqg_in = nc.dram_tensor("wqg_in", [CA, NB, 2 * CA], BF, kind="ExternalInput")
    bq_in = nc.dram_tensor("bq_in", [NB, CA], BF, kind="ExternalInput")
    wsw_in = nc.dram_tensor("wsw_in", [CA, NB, NHID], BF, kind="ExternalInput")
    wg2_in = nc.dram_tensor("wg2_in", [CA, NB, NHID], BF, kind="ExternalInput")
    wo_in = nc.dram_tensor("wo_in", [CA, NB, CA], BF, kind="ExternalInput")
    wout_in = nc.dram_tensor("wout_in", [NHID, NB, CA], BF, kind="ExternalInput")
    a_out = nc.dram_tensor("a_out", [R, CA], F32, kind="ExternalOutput")

    with tile.TileContext(nc) as tc, ExitStack() as ctx:
        const = ctx.enter_context(tc.tile_pool(name="const", bufs=1))
        ident = const.tile([128, 128], BF)
        make_identity(nc, ident)
        wz_sb = const.tile([CZ, NB * H], BF)
        nc.sync.dma_start(out=wz_sb, in_=wz_in[:])
        sel_sb = const.tile([H, NB * H], BF)
        nc.sync.dma_start(out=sel_sb, in_=sel_in[:])
        biases = const.tile([128, 1 + NB * H], F32)
        nc.vector.memset(biases[:, 0:1], EPS)
        for _bh in range(NB * H):
            nc.vector.memset(biases[:, 1 + _bh:2 + _bh], float(bias_pb[_bh]))

        pers = ctx.enter_context(tc.tile_pool(name="pers", bufs=1))
        a_sb = pers.tile([R, CA], F32)
        nc.sync.dma_start(out=a_sb, in_=a_in[:])

        # internal DRAM scratch
        dram = ctx.enter_context(tc.tile_pool(name="dram", bufs=1, space="DRAM"))
        zbeta_dr = dram.tile([NB * H, R, S], BF)

        # =========== s preprocessing (once, own rows) ===========
        with tc.tile_pool(name="sprep", bufs=1) as sp:
            s_sb = sp.tile([R, CS], F32)
            nc.sync.dma_start(out=s_sb, in_=s_in[:])
            stats = sp.tile([R, 1, 6], F32)
            mv = sp.tile([R, 2], F32)
            nc.vector.bn_stats(out=stats[:, 0, :], in_=s_sb[:])
            nc.vector.bn_aggr(out=mv, in_=stats)
            rstd = sp.tile([R, 1], F32)
            nc.scalar.activation(out=rstd, in_=mv[:, 1:2], func=ACTF.Sqrt,
                                 bias=biases[:, 0:1], scale=1.0)
            nc.vector.reciprocal(out=rstd, in_=rstd)
            s_n = sp.tile([R, CS], BF)
            nc.vector.tensor_scalar(s_n[:], s_sb[:], mv[:, 0:1], rstd[:, 0:1],
                                    OP.subtract, OP.mult)
            s_b16 = sp.tile([R, CS], BF)
            nc.vector.tensor_copy(s_b16[:], s_sb[:])

            s_nT = pers.tile([128, KT_S, 128], BF)
            sT = pers.tile([128, KT_S, 128], BF)
            ones_row = pers.tile([1, 128], BF)
            nc.vector.memset(ones_row, 1.0)
            with tc.tile_pool(name="tp_ps", bufs=2, space="PSUM") as tps:
                for kt in range(KT_S):
                    pt = tps.tile([128, 128], BF, tag="t")
                    nc.tensor.transpose(pt[:], s_n[:, bass.ts(kt, 128)], ident[:])
                    nc.scalar.copy(out=s_nT[:, kt, :], in_=pt[:])
                    pt2 = tps.tile([128, 128], BF, tag="t")
                    nc.tensor.transpose(pt2[:], s_b16[:, bass.ts(kt, 128)], ident[:])
                    nc.scalar.copy(out=sT[:, kt, :], in_=pt2[:])

            sgsb = pers.tile([R, NB, 4 * CA], BF)   # wg_a|wb_a|wg_t|wb_t
            gts = pers.tile([R, NB, 2 * CA], BF)    # gate_attn|gate_tr
            with tc.tile_pool(name="sw", bufs=2) as swp, \
                 tc.tile_pool(name="sps", bufs=3, space="PSUM") as sps:
                for b in range(NB):
                    wsn_sb = swp.tile([128, KT_S, 4 * CA], BF, tag="wsn")
                    nc.sync.dma_start(
                        out=wsn_sb,
                        in_=wsn_in[:].rearrange("(kt p) b m -> p kt b m", p=128)[:, :, b, :])
                    for m in range(6):  # 3072 / 512
                        ps = sps.tile([R, 512], F32, tag="ps")
                        for kt in range(KT_S):
                            nc.tensor.matmul(ps[:], s_nT[:, kt, :],
                                             wsn_sb[:, kt, bass.ts(m, 512)],
                                             start=(kt == 0), stop=(kt == KT_S - 1))
                        for lo_, hi_ in [(m * 512, m * 512 + 256), (m * 512 + 256, (m + 1) * 512)]:
                            mat = lo_ // CA  # 0..3
                            f = ACTF.Sigmoid if mat in (0, 2) else ACTF.Copy
                            nc.scalar.activation(
                                out=sgsb[:, b, lo_:hi_], in_=ps[:, lo_ - m * 512:hi_ - m * 512],
                                func=f, bias=0.0, scale=1.0)
                    wsr_sb = swp.tile([128, KT_S, 2 * CA], BF, tag="wsr")
                    nc.sync.dma_start(
                        out=wsr_sb,
                        in_=wsr_in[:CS].rearrange("(kt p) b m -> p kt b m", p=128)[:, :, b, :])
                    wsr_last = swp.tile([1, 2 * CA], BF, tag="wsrl")
                    nc.sync.dma_start(out=wsr_last, in_=wsr_in[CS:CS + 1, b, :])
                    for m in range(3):  # 1536 / 512
                        ps = sps.tile([R, 512], F32, tag="ps")
                        for kt in range(KT_S):
                            nc.tensor.matmul(ps[:], sT[:, kt, :],
                                             wsr_sb[:, kt, bass.ts(m, 512)],
                                             start=(kt == 0), stop=False)
                        nc.tensor.matmul(ps[:], ones_row[:],
                                         wsr_last[:, bass.ts(m, 512)],
                                         start=False, stop=True)
                        nc.scalar.activation(out=gts[:, b, bass.ts(m, 512)],
                                             in_=ps[:], func=ACTF.Sigmoid,
                                             bias=0.0, scale=1.0)

        # =========== shared per-block helpers ===========
        # Single PSUM pool, 8 banks total: ps 2x1 + t 2x1 + o 2x1 + zb5 2x1
        wpool = ctx.enter_context(tc.tile_pool(name="wpool", bufs=1))
        blk = ctx.enter_context(tc.tile_pool(name="blk", bufs=1))
        kvg = ctx.enter_context(tc.tile_pool(name="kvg", bufs=1))
        att = ctx.enter_context(tc.tile_pool(name="att", bufs=2))
        dramc = ctx.enter_context(tc.tile_pool(name="dramc", bufs=2, space="DRAM"))

        def ada_ln_own(b, pool, psp):
            """xhat of a_sb (own rows) -> ah, th (bf16) + transposed copies."""
            st3 = pool.tile([R, 3, 6], F32, tag="st3")
            for g_ in range(3):
                nc.vector.bn_stats(out=st3[:, g_, :],
                                   in_=a_sb[:, bass.ts(g_, 256)])
            mv = pool.tile([R, 2], F32, tag="mv")
            nc.vector.bn_aggr(out=mv, in_=st3)
            rstd = pool.tile([R, 1], F32, tag="rstd")
            nc.scalar.activation(out=rstd, in_=mv[:, 1:2], func=ACTF.Sqrt,
                                 bias=biases[:, 0:1], scale=1.0)
            nc.vector.reciprocal(out=rstd, in_=rstd)
            xhat = pool.tile([R, CA], BF, tag="xhat")
            nc.vector.tensor_scalar(xhat[:], a_sb[:], mv[:, 0:1], rstd[:, 0:1],
                                    OP.subtract, OP.mult)
            ah = pool.tile([R, CA], BF, tag="ah")
            nc.vector.tensor_tensor(ah[:], xhat[:], sgsb[:, b, 0:CA], OP.mult)
            nc.vector.tensor_tensor(ah[:], ah[:], sgsb[:, b, CA:2 * CA], OP.add)
            th = pool.tile([R, CA], BF, tag="th")
            nc.vector.tensor_tensor(th[:], xhat[:], sgsb[:, b, 2 * CA:3 * CA], OP.mult)
            nc.vector.tensor_tensor(th[:], th[:], sgsb[:, b, 3 * CA:4 * CA], OP.add)
            ahT = pool.tile([128, KT_A, 128], BF, tag="ahT")
            thT = pool.tile([128, KT_A, 128], BF, tag="thT")
            for src, dst in ((ah, ahT), (th, thT)):
                pt = psp.tile([128, JT, 128], BF, tag="t8")
                for kt in range(KT_A):
                    nc.tensor.transpose(pt[:, kt, :], src[:, bass.ts(kt, 128)],
                                        ident[:])
                if src is ah:
                    nc.scalar.copy(out=dst[:], in_=pt[:, 0:KT_A, :])
                else:
                    nc.vector.tensor_copy(dst[:], pt[:, 0:KT_A, :])
            return ahT, thT

        def kv_pack_gather(b, ahT, pool, psp):
            """k/v for own rows; head-padded kT halves + v rows gathered."""
            wkv_sb = wpool.tile([128, KT_A, 2 * CA], BF, tag="w1536")
            nc.sync.dma_start(
                out=wkv_sb,
                in_=wkv_in[:].rearrange("(kt p) b m -> p kt b m", p=128)[:, :, b, :])
            k_own = pool.tile([R, CA], BF, tag="k_own")
            v_own = pool.tile([R, CA], BF, tag="v_own")
            for m in range(3):  # 1536/512
                ps = psp.tile([R, 512], F32, tag="ps")
                for kt in range(KT_A):
                    nc.tensor.matmul(ps[:], ahT[:, kt, :],
                                     wkv_sb[:, kt, bass.ts(m, 512)],
                                     start=(kt == 0), stop=(kt == KT_A - 1))
                if m == 0:
                    nc.scalar.copy(out=k_own[:, 0:512], in_=ps[:])
                elif m == 1:
                    nc.scalar.copy(out=k_own[:, 512:768], in_=ps[:, 0:256])
                    nc.scalar.copy(out=v_own[:, 0:256], in_=ps[:, 256:512])
                else:
                    nc.scalar.copy(out=v_own[:, 256:768], in_=ps[:])
            # per-head transposes into 64-aligned psum slots (pad 48->64)
            pt = psp.tile([128, JT, 128], BF, tag="t8")
            for h in range(H):
                nc.tensor.transpose(pt[64 * (h % 2):64 * (h % 2) + D, h // 2, :],
                                    k_own[:, h * D:(h + 1) * D], ident[:])
            kT_e = pool.tile([D, 8, 128], BF, tag="kT_e")
            kT_o = pool.tile([D, 8, 128], BF, tag="kT_o")
            nc.vector.tensor_copy(kT_e[:], pt[0:D, :, :])
            nc.vector.tensor_copy(kT_o[:], pt[64:64 + D, :, :])
            kv_inb = dramc.tile([KV_TOT], BF, tag="kvin")
            kv_outb = dramc.tile([NCORE * KV_TOT], BF, tag="kvout",
                                 addr_space="Shared")
            KH = KV_K // 2
            nc.gpsimd.dma_start(
                out=kv_inb[0:KH].rearrange("(d x) -> d x", d=D),
                in_=kT_e[:].rearrange("d t j -> d (t j)"))
            nc.gpsimd.dma_start(
                out=kv_inb[KH:KV_K].rearrange("(d x) -> d x", d=D),
                in_=kT_o[:].rearrange("d t j -> d (t j)"))
            nc.gpsimd.dma_start(
                out=kv_inb[KV_K:].rearrange("(j c) -> j c", j=R), in_=v_own[:])
            nc.gpsimd.collective_compute(
                "AllGather", OP.bypass,
                replica_groups=[list(range(NCORE))],
                ins=[kv_inb[:].opt()], outs=[kv_outb[:].opt()])
            return kv_outb

        def kv_unpack(kv_outb):
            kv_outr = kv_outb[:].rearrange("(r x) -> r x", r=NCORE)
            KH = KV_K // 2
            kT_fe = kvg.tile([D, 8, NCORE, 128], BF, tag="kTfe")
            nc.gpsimd.dma_start(
                out=kT_fe,
                in_=kv_outr[:, 0:KH].rearrange("r (d t j) -> d t r j", d=D, t=8))
            kT_fo = kvg.tile([D, 8, NCORE, 128], BF, tag="kTfo")
            nc.gpsimd.dma_start(
                out=kT_fo,
                in_=kv_outr[:, KH:KV_K].rearrange(
                    "r (d t j) -> d t r j", d=D, t=8))
            v_full = kvg.tile([128, NCORE, CA], BF, tag="vf")
            nc.gpsimd.dma_start(
                out=v_full,
                in_=kv_outr[:, KV_K:].rearrange("r (j c) -> j r c", j=R))
            return (kT_fe, kT_fo), v_full

        def qg_own(b, ahT, pool, psp):
            """q (row + bias, then transposed to even/odd head tiles), gate g."""
            wqg_sb = wpool.tile([128, KT_A, 2 * CA], BF, tag="w1536")
            nc.sync.dma_start(
                out=wqg_sb,
                in_=wqg_in[:].rearrange("(kt p) b m -> p kt b m", p=128)[:, :, b, :])
            bq_row = pool.tile([1, CA], BF, tag="bq_row")
            nc.sync.dma_start(out=bq_row, in_=bq_in[b:b + 1, :])
            bq_mat = pool.tile([R, CA], BF, tag="bq_mat")
            nc.gpsimd.partition_broadcast(bq_mat[:], bq_row[:])
            q_own = pool.tile([R, CA], BF, tag="k_own", name="q_own")
            g_own = pool.tile([R, CA], BF, tag="g_own")
            for m in range(3):
                ps = psp.tile([R, 512], F32, tag="ps")
                for kt in range(KT_A):
                    nc.tensor.matmul(ps[:], ahT[:, kt, :],
                                     wqg_sb[:, kt, bass.ts(m, 512)],
                                     start=(kt == 0), stop=(kt == KT_A - 1))
                if m == 0:
                    nc.vector.tensor_tensor(q_own[:, 0:512], ps[:],
                                            bq_mat[:, 0:512], OP.add)
                elif m == 1:
                    nc.vector.tensor_tensor(q_own[:, 512:768], ps[:, 0:256],
                                            bq_mat[:, 512:768], OP.add)
                    nc.scalar.activation(out=g_own[:, 0:256], in_=ps[:, 256:512],
                                         func=ACTF.Sigmoid, bias=0.0, scale=1.0)
                else:
                    nc.scalar.activation(out=g_own[:, 256:768], in_=ps[:],
                                         func=ACTF.Sigmoid, bias=0.0, scale=1.0)
            pt = psp.tile([128, JT, 128], BF, tag="t8")
            for h in range(H):
                nc.tensor.transpose(pt[64 * (h % 2):64 * (h % 2) + D, h // 2, :],
                                    q_own[:, h * D:(h + 1) * D], ident[:])
            qT_e = pool.tile([D, 8, 128], BF, tag="qT_e")
            qT_o = pool.tile([D, 8, 128], BF, tag="qT_o")
            nc.scalar.copy(out=qT_e[:], in_=pt[0:D, :, :])
            nc.scalar.copy(out=qT_o[:], in_=pt[64:64 + D, :, :])
            return (qT_e, qT_o), g_own

        def transition(b, thT, pool, psp):
            """hidT = silu(th@wsw)^T * (th@wg2)^T in [hid, i] tiles; then
            tr = gate_tr * (hidden @ w_out) accumulated later."""
            wsw_sb = wpool.tile([128, KT_A, NHID], BF, tag="wbig")
            nc.sync.dma_start(
                out=wsw_sb,
                in_=wsw_in[:].rearrange("(kt p) b m -> p kt b m", p=128)[:, :, b, :])
            wg2_sb = wpool.tile([128, KT_A, NHID], BF, tag="w1536")
            nc.sync.dma_start(
                out=wg2_sb,
                in_=wg2_in[:].rearrange("(kt p) b m -> p kt b m", p=128)[:, :, b, :])
            # hidden row-form [R, NHID] via 3 psum chunks; then transpose.
            sw_row = pool.tile([R, NHID], BF, tag="sw_row")
            for m in range(3):
                ps = psp.tile([R, 512], F32, tag="ps")
                for kt in range(KT_A):
                    nc.tensor.matmul(ps[:], thT[:, kt, :],
                                     wsw_sb[:, kt, bass.ts(m, 512)],
                                     start=(kt == 0), stop=(kt == KT_A - 1))
                nc.scalar.activation(out=sw_row[:, bass.ts(m, 512)], in_=ps[:],
                                     func=ACTF.Silu, bias=0.0, scale=1.0)
            hid_row = pool.tile([R, NHID], BF, tag="hid_row")
            for m in range(3):
                ps = psp.tile([R, 512], F32, tag="ps")
                for kt in range(KT_A):
                    nc.tensor.matmul(ps[:], thT[:, kt, :],
                                     wg2_sb[:, kt, bass.ts(m, 512)],
                                     start=(kt == 0), stop=(kt == KT_A - 1))
                g2c = pool.tile([R, 512], BF, tag="g2c")
                nc.scalar.copy(out=g2c, in_=ps[:])
                nc.vector.tensor_tensor(hid_row[:, bass.ts(m, 512)],
                                        sw_row[:, bass.ts(m, 512)], g2c[:],
                                        OP.mult)
            hidT = pool.tile([128, KT_H, 128], BF, tag="hidT")
            for g_ in range(2):
                pt = psp.tile([128, JT, 128], BF, tag="t8")
                for q in range(6):
                    kt = g_ * 6 + q
                    nc.tensor.transpose(pt[:, q, :], hid_row[:, bass.ts(kt, 128)],
                                        ident[:])
                if g_ % 2 == 0:
                    nc.scalar.copy(out=hidT[:, 0:6, :], in_=pt[:, 0:6, :])
                else:
                    nc.vector.tensor_copy(hidT[:, 6:12, :], pt[:, 0:6, :])
            return hidT

        def attention(b, qT, g_own, kT_full, v_full, pool, psp):
            """Row-softmax attention with unnormalized exp; 1/sum folded into
            the g*o stage. Returns go_row [R, CA] bf16."""
            qT_e, qT_o = qT
            kT_fe, kT_fo = kT_full
            sums = pool.tile([R, H, 2], F32, tag="sums")
            go_row = pool.tile([R, CA], BF, tag="v_own", name="go_row")
            for h in range(H):
                qT_h = (qT_e if h % 2 == 0 else qT_o)[:, h // 2, :]
                kT_h = (kT_fe if h % 2 == 0 else kT_fo)[:, h // 2, :, :]
                attn = att.tile([R, S], BF, tag="attn")
                zb_t = att.tile([R, S], BF, tag="zbt")
                nc.gpsimd.dma_start(out=zb_t, in_=zbeta_dr[b * H + h, :, :])
                bh_ = 1 + b * H + h
                ps_s = psp.tile([R, S], F32, tag="ps2")
                for jc in range(2):
                    nc.tensor.matmul(
                        ps_s[:, bass.ts(jc, 512)], qT_h,
                        kT_h.rearrange("d r j -> d (r j)")[:, bass.ts(jc, 512)],
                        start=True, stop=True)
                nc.vector.tensor_tensor(ps_s[:], ps_s[:], zb_t[:], OP.add)
                nc.scalar.activation(out=attn, in_=ps_s[:], func=ACTF.Exp,
                                     bias=biases[:, bh_:bh_ + 1], scale=1.0,
                                     accum_out=sums[:, h, 0:1])
                attnT = att.tile([128, JT, 128], BF, tag="attnT")
                pt = psp.tile([128, JT, 128], BF, tag="t8")
                for jt in range(JT):
                    nc.tensor.transpose(pt[:, jt, :],
                                        attn[:, bass.ts(jt, 128)], ident[:])
                if h % 2 == 0:
                    nc.vector.tensor_copy(attnT[:], pt[:])
                else:
                    nc.scalar.copy(out=attnT[:], in_=pt[:])
                # o[i, d] for this head: lhsT = attnT tiles, rhs = v cols
                ps_o = psp.tile([R, D], F32, tag="ps")
                for jt in range(JT):
                    nc.tensor.matmul(ps_o[:], attnT[:, jt, :],
                                     v_full[:, jt, h * D:(h + 1) * D],
                                     start=(jt == 0), stop=(jt == JT - 1))
                rec = att.tile([R, 1], F32, tag="rec")
                nc.vector.reciprocal(out=rec, in_=sums[:, h, 0:1])
                tmp = att.tile([R, D], F32, tag="tmp")
                nc.vector.tensor_scalar(tmp[:], ps_o[:], rec[:, 0:1], None,
                                        OP.mult)
                nc.vector.tensor_tensor(go_row[:, h * D:(h + 1) * D], tmp[:],
                                        g_own[:, h * D:(h + 1) * D], OP.mult)
            return go_row

        def out_proj_and_update(b, go_row, hidT, pool, psp):
            """a = gate_a*(g*o)@wo + gate_t*(hidden@w_out), written to a_sb."""
            wo_sb = wpool.tile([128, KT_A, CA], BF, tag="w1536",
                               padded_shape=[128, KT_A, 2 * CA])
            nc.sync.dma_start(
                out=wo_sb,
                in_=wo_in[:].rearrange("(kt p) b m -> p kt b m", p=128)[:, :, b, :])
            wout_sb = wpool.tile([128, KT_H, CA], BF, tag="wbig",
                                 padded_shape=[128, KT_H, CA])
            nc.sync.dma_start(
                out=wout_sb,
                in_=wout_in[:].rearrange("(kt p) b m -> p kt b m", p=128)[:, :, b, :])
            goT = pool.tile([128, KT_A, 128], BF, tag="goT")
            pt = psp.tile([128, JT, 128], BF, tag="t8")
            for kt in range(KT_A):
                nc.tensor.transpose(pt[:, kt, :], go_row[:, bass.ts(kt, 128)],
                                    ident[:])
            nc.scalar.copy(out=goT[:], in_=pt[:, 0:KT_A, :])
            for m in range(2):
                n0, n1 = (0, 512) if m == 0 else (512, 768)
                ps = psp.tile([R, 512], F32, tag="ps")
                for kt in range(KT_A):
                    nc.tensor.matmul(ps[:, 0:n1 - n0], goT[:, kt, :],
                                     wo_sb[:, kt, n0:n1],
                                     start=(kt == 0), stop=(kt == KT_A - 1))
                batt = pool.tile([R, 512], F32, tag="batt")
                nc.vector.tensor_tensor(batt[:, 0:n1 - n0], ps[:, 0:n1 - n0],
                                        gts[:, b, n0:n1], OP.mult)
                ps2 = psp.tile([R, 512], F32, tag="ps")
                for kt in range(KT_H):
                    nc.tensor.matmul(ps2[:, 0:n1 - n0], hidT[:, kt, :],
                                     wout_sb[:, kt, n0:n1],
                                     start=(kt == 0), stop=(kt == KT_H - 1))
                tr = pool.tile([R, 512], F32, tag="tr")
                nc.vector.tensor_tensor(tr[:, 0:n1 - n0], ps2[:, 0:n1 - n0],
                                        gts[:, b, CA + n0:CA + n1], OP.mult)
                nc.vector.tensor_tensor(a_sb[:, n0:n1], batt[:, 0:n1 - n0],
                                        tr[:, 0:n1 - n0], OP.add)

        # =========== block 0 front (kv gather launches before z-prep) ===========
        psA = ctx.enter_context(tc.tile_pool(name="psA", bufs=2, space="PSUM"))
        ahT0, thT0 = ada_ln_own(0, blk, psA)
        kv_out0 = kv_pack_gather(0, ahT0, blk, psA)
        qT0, g0 = qg_own(0, ahT0, blk, psA)
        hidT0 = transition(0, thT0, blk, psA)

        # =========== z preprocessing (once), groups of 4 i-rows ===========
        with tc.tile_pool(name="zslab", bufs=3) as zsl, \
             tc.tile_pool(name="zsm", bufs=3) as zsm:
            for i4 in range(R // 4):
                zts = []
                st8 = zsm.tile([128, 4, JT, 6], F32, tag="st", bufs=2)
                for di in range(4):
                    zt = zsl.tile([128, JT, CZ], BF, tag="z", bufs=6)
                    nc.sync.dma_start(out=zt[:].rearrange("p jt c -> p (jt c)"),
                                      in_=z_in[i4 * 4 + di])
                    zts.append(zt)
                    for jt in range(JT):
                        nc.vector.bn_stats(out=st8[:, di, jt, :], in_=zt[:, jt, :])
                # pooled even/odd moments, batched over 4 i-rows (on Pool):
                # mean = (m_e+m_o)/2; var = (M2_e+M2_o)/CZ + ((m_e-m_o)/2)^2
                J4 = 4 * JT
                s1 = st8[:].rearrange("p di jt s -> p (di jt) s")
                mrow = zsm.tile([128, 4, JT], F32, tag="mrow", bufs=2)
                m2 = mrow[:].rearrange("p di jt -> p (di jt)")
                nc.gpsimd.tensor_tensor(m2, s1[:, :, 1], s1[:, :, 4], OP.add)
                nc.gpsimd.tensor_scalar(m2, m2, 0.5, None, OP.mult)
                dm = zsm.tile([128, J4], F32, tag="dm", bufs=2)
                nc.gpsimd.tensor_tensor(dm[:], s1[:, :, 1], s1[:, :, 4],
                                        OP.subtract)
                nc.gpsimd.tensor_tensor(dm[:], dm[:], dm[:], OP.mult)
                nc.gpsimd.tensor_scalar(dm[:], dm[:], 0.25, None, OP.mult)
                var = zsm.tile([128, J4], F32, tag="var", bufs=2)
                nc.gpsimd.tensor_tensor(var[:], s1[:, :, 2], s1[:, :, 5], OP.add)
                nc.gpsimd.tensor_scalar(var[:], var[:], 1.0 / CZ, None, OP.mult)
                nc.gpsimd.tensor_tensor(var[:], var[:], dm[:], OP.add)
                rst = zsm.tile([128, 4, JT], F32, tag="rst", bufs=2)
                r2 = rst[:].rearrange("p di jt -> p (di jt)")
                nc.scalar.activation(out=r2, in_=var[:], func=ACTF.Sqrt,
                                     bias=biases[:, 0:1], scale=1.0)
                nc.vector.reciprocal(out=r2, in_=r2)
                for dp in range(2):  # pairs of i-rows share one zb psum tile
                    zb = psA.tile([128, S], F32, tag="ps2")
                    for dj in range(2):
                        di = dp * 2 + dj
                        i = i4 * 4 + di
                        zt = zts[di]
                        bsel = zsl.tile([H, S], BF, tag="bsel")
                        nc.sync.dma_start(out=bsel, in_=betaT_in[:, i, :])
                        zh = zsm.tile([128, JT, CZ], BF, tag="zh", bufs=2)
                        eng = nc.vector if di == 0 else nc.gpsimd
                        for jt in range(JT):
                            eng.tensor_scalar(zh[:, jt, :], zt[:, jt, :],
                                              mrow[:, di, jt:jt + 1],
                                              rst[:, di, jt:jt + 1],
                                              OP.subtract, OP.mult)
                        zhT = zsm.tile([128, JT, 128], BF, tag="zhT", bufs=2)
                        pt = psA.tile([128, JT, 128], BF, tag="t8")
                        for jt in range(JT):
                            nc.tensor.transpose(pt[:, jt, :], zh[:, jt, :],
                                                ident[:])
                        if di == 0:
                            nc.vector.tensor_copy(zhT[:], pt[:])
                        else:
                            nc.scalar.copy(out=zhT[:], in_=pt[:])
                        zrow = zb[64 * dj:64 * dj + 64, :]
                        for jc in range(2):
                            nc.tensor.matmul(zrow[:, bass.ts(jc, 512)], wz_sb[:],
                                             zhT[:].rearrange("p jt j -> p (jt j)")[:, bass.ts(jc, 512)],
                                             start=True, stop=False)
                            nc.tensor.matmul(zrow[:, bass.ts(jc, 512)], sel_sb[:],
                                             bsel[:, bass.ts(jc, 512)],
                                             start=False, stop=True)
                    zbs = zsm.tile([128, S], BF, tag="zbs", bufs=2)
                    nc.scalar.copy(out=zbs, in_=zb[:])
                    i0 = i4 * 4 + dp * 2
                    nc.sync.dma_start(out=zbeta_dr[:, i0, :], in_=zbs[0:64, :])
                    nc.sync.dma_start(out=zbeta_dr[:, i0 + 1, :],
                                      in_=zbs[64:128, :])

        # =========== block loop ===========
        for b in range(NB):
            if b == 0:
                kv_outb, qT, g_own, hidT = kv_out0, qT0, g0, hidT0
            else:
                ahT, thT = ada_ln_own(b, blk, psA)
                kv_outb = kv_pack_gather(b, ahT, blk, psA)
                qT, g_own = qg_own(b, ahT, blk, psA)
                hidT = transition(b, thT, blk, psA)
            kT_full, v_full = kv_unpack(kv_outb)
            go_row = attention(b, qT, g_own, kT_full, v_full, blk, psA)
            out_proj_and_update(b, go_row, hidT, blk, psA)

        nc.sync.dma_start(out=a_out[:], in_=a_sb[:])

    nc.finalize()
    return nc


def _prep_inputs(a, s, z, beta, ln_s_w_attn, wg_attn, wb_attn, wq, bq, wk, wv,
                 ln_z_w, ln_z_b, wpb, wgate, wo, wsg_attn, bsg_attn,
                 ln_s_w_tr, wg_tr, wb_tr, w_swish, w_gate2, wsg_tr, bsg_tr, w_out):
    bf = ml_dtypes.bfloat16
    f32 = np.float32
    scale = 1.0 / np.sqrt(np.float32(D))

    # folded weights (shared across cores)
    wz = np.concatenate([ln_z_w[i][:, None] * wpb[i] for i in range(NB)],
                        axis=1).astype(bf)                       # [CZ, NB*H]
    bias_pb = np.concatenate([ln_z_b[i] @ wpb[i] for i in range(NB)])  # [NB*H]
    sel = np.tile(np.eye(H, dtype=np.float32), (1, NB)).astype(bf)  # [H, NB*H]
    wsn = np.stack([np.concatenate(
        [ln_s_w_attn[i][:, None] * wg_attn[i], ln_s_w_attn[i][:, None] * wb_attn[i],
         ln_s_w_tr[i][:, None] * wg_tr[i], ln_s_w_tr[i][:, None] * wb_tr[i]],
        axis=1) for i in range(NB)], axis=1).astype(bf)          # [CS, NB, 4CA]
    wsr = np.stack([np.concatenate(
        [np.concatenate([wsg_attn[i], bsg_attn[i][None, :]], 0),
         np.concatenate([wsg_tr[i], bsg_tr[i][None, :]], 0)], axis=1)
        for i in range(NB)], axis=1).astype(bf)                  # [CS+1, NB, 2CA]
    wkv = np.stack([np.concatenate([wk[i], wv[i]], 1) for i in range(NB)],
                   axis=1).astype(bf)                            # [CA, NB, 2CA]
    wqg = np.stack([np.concatenate([wq[i] * scale, wgate[i]], 1)
                    for i in range(NB)], axis=1).astype(bf)
    bqe = (bq * scale).astype(bf)                               # [NB, CA]
    wsw = np.stack([w_swish[i] for i in range(NB)], axis=1).astype(bf)
    wg2 = np.stack([w_gate2[i] for i in range(NB)], axis=1).astype(bf)
    wob = np.stack([wo[i] for i in range(NB)], axis=1).astype(bf)
    wout = np.stack([w_out[i] for i in range(NB)], axis=1).astype(bf)

    shared = dict(wz_in=np.ascontiguousarray(wz),
                  sel_in=np.ascontiguousarray(sel),
                  wsn_in=np.ascontiguousarray(wsn),
                  wsr_in=np.ascontiguousarray(wsr),
                  wkv_in=np.ascontiguousarray(wkv),
                  wqg_in=np.ascontiguousarray(wqg),
                  bq_in=np.ascontiguousarray(bqe),
                  wsw_in=np.ascontiguousarray(wsw),
                  wg2_in=np.ascontiguousarray(wg2),
                  wo_in=np.ascontiguousarray(wob),
                  wout_in=np.ascontiguousarray(wout))

    a2 = a.reshape(S, CA).astype(f32)
    s2 = s.reshape(S, CS).astype(f32)
    z2 = z.reshape(S, S, CZ).astype(bf)
    betaT = np.ascontiguousarray(
        beta.reshape(S, S, H).transpose(2, 0, 1)).astype(bf)     # [H, S, S]

    in_maps = []
    for c in range(NCORE):
        rows = slice(c * R, (c + 1) * R)
        m = dict(shared)
        m["a_in"] = np.ascontiguousarray(a2[rows])
        m["s_in"] = np.ascontiguousarray(s2[rows])
        # [i, jp, jt, c] so SBUF partition lines are (jt, c) = 2KB contiguous
        zj = z2[rows].reshape(R, JT, 128, CZ).transpose(0, 2, 1, 3)
        m["z_in"] = np.ascontiguousarray(zj.reshape(R, 128, JT * CZ))
        m["betaT_in"] = np.ascontiguousarray(betaT[:, rows, :])
        in_maps.append(m)
    return in_maps, [float(x) for x in bias_pb]


_CACHE = {}


def kernel(**inputs):
    inputs = {k: np.asarray(v) for k, v in inputs.items()}
    in_maps, bias_pb = _prep_inputs(**inputs)
    key = tuple(bias_pb)
    if key not in _CACHE:
        _CACHE.clear()
        _CACHE[key] = build_program(bias_pb)
    nc = _CACHE[key]
    res = run_bass_kernel_spmd(nc, in_maps, core_ids=list(range(NCORE)),
                               trace=False)
    out = np.concatenate([res.results[c]["a_out"] for c in range(NCORE)], axis=0)
    return out.reshape(1, S, CA).astype(np.float32)


if __name__ == "__main__":
    import reference
    ins = {k: np.asarray(v) for k, v in reference.setup_inputs().items()}
    exp = np.asarray(reference.reference(**ins))
    act = kernel(**ins)
    err = np.abs(act - exp).max() / (np.abs(exp).max() + 1e-9)
    print("rel err:", err)


# revision 61
# speedup vs baseline: 1.2051x; 1.0475x over previous
"""DiffusionTransformer (AF3-style) Trainium2 kernel, 8-core SPMD, v2.

Sharding: sequence-parallel over rows (queries). Each core owns R=128 rows of
a / z / beta. Per block, k/v are computed on local rows (row-form matmuls),
packed, and AllGathered (bf16, ~384KB per rank); block-0's gather overlaps
the one-time z preprocessing. zbeta (pair bias LN(z)@wpb + beta, all 4
blocks) is computed in one pass over z into per-core DRAM in [bh, i, j]
layout and consumed as [i, j] strips per (block, head).
"""
import numpy as np
import ml_dtypes
from contextlib import ExitStack

import concourse.bass as bass
import concourse.mybir as mybir
import concourse.tile as tile
from concourse import bacc
from concourse.bass_utils import run_bass_kernel_spmd
from concourse.masks import make_identity

NB, H, S, CA, CS, CZ = 4, 16, 1024, 768, 384, 128
D = CA // H            # 48
NCORE = 8
R = S // NCORE         # 128 rows per core
NHID = 2 * CA          # 1536
EPS = 1e-5
BF = mybir.dt.bfloat16
F32 = mybir.dt.float32
AX = mybir.AxisListType
OP = mybir.AluOpType
ACTF = mybir.ActivationFunctionType
JT = S // 128          # 8 j-tiles
KT_A = CA // 128       # 6 k-tiles over c_a
KT_S = CS // 128       # 3 k-tiles over c_s
KT_H = NHID // 128     # 12 k-tiles over n_hidden
KV_TOT = R * 2 * CA    # flat bf16 elems gathered per rank (kT tiles + v rows)
KV_K = R * CA          # kT part


def _head_pieces():
    """Split each head's d-range [48h, 48h+48) at 128-tile boundaries.

    Returns per-head list of (kt, p0, plen, d0) with kt*128+p0 == 48h+d0.
    """
    out = []
    for h in range(H):
        lo, hi = h * D, (h + 1) * D
        pieces = []
        while lo < hi:
            kt, p0 = lo // 128, lo % 128
            plen = min(hi - lo, 128 - p0)
            pieces.append((kt, p0, plen, lo - h * D))
            lo += plen
        out.append(pieces)
    return out


HP = _head_pieces()


def build_program(bias_pb):
    nc = bacc.Bacc("TRN2", target_bir_lowering=False, debug=False,
                   num_devices=NCORE)

    # ---------------- I/O ----------------
    a_in = nc.dram_tensor("a_in", [R, CA], F32, kind="ExternalInput")
    s_in = nc.dram_tensor("s_in", [R, CS], F32, kind="ExternalInput")
    # z in [i][jp][jt*c] layout: partition lines are 2KB contiguous
    z_in = nc.dram_tensor("z_in", [R, 128, JT * CZ], BF, kind="ExternalInput")
    betaT_in = nc.dram_tensor("betaT_in", [H, R, S], BF, kind="ExternalInput")
    wz_in = nc.dram_tensor("wz_in", [CZ, NB * H], BF, kind="ExternalInput")
    sel_in = nc.dram_tensor("sel_in", [H, NB * H], BF, kind="ExternalInput")
    wsn_in = nc.dram_tensor("wsn_in", [CS, NB, 4 * CA], BF, kind="ExternalInput")
    wsr_in = nc.dram_tensor("wsr_in", [CS + 1, NB, 2 * CA], BF, kind="ExternalInput")
    wkv_in = nc.dram_tensor("wkv_in", [CA, NB, 2 * CA], BF, kind="ExternalInput")
    wqg_in = nc.dram_tensor("wqg_in", [CA, NB, 2 * CA], BF, kind="ExternalInput")
    bq_in = nc.dram_tensor("bq_in", [NB, CA], BF, kind="ExternalInput")
    wsw_in = nc.dram_tensor("wsw_in", [CA, NB, NHID], BF, kind="ExternalInput")
    wg2_in = nc.dram_tensor("wg2_in", [CA, NB, NHID], BF, kind="ExternalInput")
    wo_in = nc.dram_tensor("wo_in", [CA, NB, CA], BF, kind="ExternalInput")
    wout_in = nc.dram_tensor("wout_in", [NHID, NB, CA], BF, kind="ExternalInput")
    a_out = nc.dram_tensor("a_out", [R, CA], F32, kind="ExternalOutput")

    with tile.TileContext(nc) as tc, ExitStack() as ctx:
        const = ctx.enter_context(tc.tile_pool(name="const", bufs=1))
        ident = const.tile([128, 128], BF)
        make_identity(nc, ident)
        wz_sb = const.tile([CZ, NB * H], BF)
        nc.sync.dma_start(out=wz_sb, in_=wz_in[:])
        sel_sb = const.tile([H, NB * H], BF)
        nc.sync.dma_start(out=sel_sb, in_=sel_in[:])
        biases = const.tile([128, 1 + NB * H], F32)
        nc.vector.memset(biases[:, 0:1], EPS)
        for _bh in range(NB * H):
            nc.vector.memset(biases[:, 1 + _bh:2 + _bh], float(bias_pb[_bh]))

        pers = ctx.enter_context(tc.tile_pool(name="pers", bufs=1))
        a_sb = pers.tile([R, CA], F32)
        nc.sync.dma_start(out=a_sb, in_=a_in[:])

        # internal DRAM scratch
        dram = ctx.enter_context(tc.tile_pool(name="dram", bufs=1, space="DRAM"))
        zbeta_dr = dram.tile([NB * H, R, S], BF)

        # =========== s preprocessing (once, own rows) ===========
        with tc.tile_pool(name="sprep", bufs=1) as sp:
            s_sb = sp.tile([R, CS], F32)
            nc.sync.dma_start(out=s_sb, in_=s_in[:])
            stats = sp.tile([R, 1, 6], F32)
            mv = sp.tile([R, 2], F32)
            nc.vector.bn_stats(out=stats[:, 0, :], in_=s_sb[:])
            nc.vector.bn_aggr(out=mv, in_=stats)
            rstd = sp.tile([R, 1], F32)
            nc.scalar.activation(out=rstd, in_=mv[:, 1:2], func=ACTF.Sqrt,
                                 bias=biases[:, 0:1], scale=1.0)
            nc.vector.reciprocal(out=rstd, in_=rstd)
            s_n = sp.tile([R, CS], BF)
            nc.vector.tensor_scalar(s_n[:], s_sb[:], mv[:, 0:1], rstd[:, 0:1],
                                    OP.subtract, OP.mult)
            s_b16 = sp.tile([R, CS], BF)
            nc.vector.tensor_copy(s_b16[:], s_sb[:])

            s_nT = pers.tile([128, KT_S, 128], BF)
            sT = pers.tile([128, KT_S, 128], BF)
            ones_row = pers.tile([1, 128], BF)
            nc.vector.memset(ones_row, 1.0)
            with tc.tile_pool(name="tp_ps", bufs=2, space="PSUM") as tps:
                for kt in range(KT_S):
                    pt = tps.tile([128, 128], BF, tag="t")
                    nc.tensor.transpose(pt[:], s_n[:, bass.ts(kt, 128)], ident[:])
                    nc.scalar.copy(out=s_nT[:, kt, :], in_=pt[:])
                    pt2 = tps.tile([128, 128], BF, tag="t")
                    nc.tensor.transpose(pt2[:], s_b16[:, bass.ts(kt, 128)], ident[:])
                    nc.scalar.copy(out=sT[:, kt, :], in_=pt2[:])

            sgsb = pers.tile([R, NB, 4 * CA], BF)   # wg_a|wb_a|wg_t|wb_t
            gts = pers.tile([R, NB, 2 * CA], BF)    # gate_attn|gate_tr
            with tc.tile_pool(name="sw", bufs=2) as swp, \
                 tc.tile_pool(name="sps", bufs=3, space="PSUM") as sps:
                for b in range(NB):
                    wsn_sb = swp.tile([128, KT_S, 4 * CA], BF, tag="wsn")
                    nc.sync.dma_start(
                        out=wsn_sb,
                        in_=wsn_in[:].rearrange("(kt p) b m -> p kt b m", p=128)[:, :, b, :])
                    for m in range(6):  # 3072 / 512
                        ps = sps.tile([R, 512], F32, tag="ps")
                        for kt in range(KT_S):
                            nc.tensor.matmul(ps[:], s_nT[:, kt, :],
                                             wsn_sb[:, kt, bass.ts(m, 512)],
                                             start=(kt == 0), stop=(kt == KT_S - 1))
                        for lo_, hi_ in [(m * 512, m * 512 + 256), (m * 512 + 256, (m + 1) * 512)]:
                            mat = lo_ // CA  # 0..3
                            f = ACTF.Sigmoid if mat in (0, 2) else ACTF.Copy
                            nc.scalar.activation(
                                out=sgsb[:, b, lo_:hi_], in_=ps[:, lo_ - m * 512:hi_ - m * 512],
                                func=f, bias=0.0, scale=1.0)
                    wsr_sb = swp.tile([128, KT_S, 2 * CA], BF, tag="wsr")
                    nc.sync.dma_start(
                        out=wsr_sb,
                        in_=wsr_in[:CS].rearrange("(kt p) b m -> p kt b m", p=128)[:, :, b, :])
                    wsr_last = swp.tile([1, 2 * CA], BF, tag="wsrl")
                    nc.sync.dma_start(out=wsr_last, in_=wsr_in[CS:CS + 1, b, :])
                    for m in range(3):  # 1536 / 512
                        ps = sps.tile([R, 512], F32, tag="ps")
                        for kt in range(KT_S):
                            nc.tensor.matmul(ps[:], sT[:, kt, :],
                                             wsr_sb[:, kt, bass.ts(m, 512)],
                                             start=(kt == 0), stop=False)
                        nc.tensor.matmul(ps[:], ones_row[:],
                                         wsr_last[:, bass.ts(m, 512)],
                                         start=False, stop=True)
                        nc.scalar.activation(out=gts[:, b, bass.ts(m, 512)],
                                             in_=ps[:], func=ACTF.Sigmoid,
                                             bias=0.0, scale=1.0)

        # =========== shared per-block helpers ===========
        # Single PSUM pool, 8 banks total: ps 2x1 + t 2x1 + o 2x1 + zb5 2x1
        wpool = ctx.enter_context(tc.tile_pool(name="wpool", bufs=1))
        blk = ctx.enter_context(tc.tile_pool(name="blk", bufs=1))
        kvg = ctx.enter_context(tc.tile_pool(name="kvg", bufs=1))
        att = ctx.enter_context(tc.tile_pool(name="att", bufs=2))
        dramc = ctx.enter_context(tc.tile_pool(name="dramc", bufs=2, space="DRAM"))

        def ada_ln_own(b, pool, psp):
            """xhat of a_sb (own rows) -> ah, th (bf16) + transposed copies."""
            st3 = pool.tile([R, 3, 6], F32, tag="st3")
            for g_ in range(3):
                nc.vector.bn_stats(out=st3[:, g_, :],
                                   in_=a_sb[:, bass.ts(g_, 256)])
            mv = pool.tile([R, 2], F32, tag="mv")
            nc.vector.bn_aggr(out=mv, in_=st3)
            rstd = pool.tile([R, 1], F32, tag="rstd")
            nc.scalar.activation(out=rstd, in_=mv[:, 1:2], func=ACTF.Sqrt,
                                 bias=biases[:, 0:1], scale=1.0)
            nc.vector.reciprocal(out=rstd, in_=rstd)
            xhat = pool.tile([R, CA], BF, tag="xhat")
            nc.vector.tensor_scalar(xhat[:], a_sb[:], mv[:, 0:1], rstd[:, 0:1],
                                    OP.subtract, OP.mult)
            ah = pool.tile([R, CA], BF, tag="ah")
            nc.vector.tensor_tensor(ah[:], xhat[:], sgsb[:, b, 0:CA], OP.mult)
            nc.vector.tensor_tensor(ah[:], ah[:], sgsb[:, b, CA:2 * CA], OP.add)
            th = pool.tile([R, CA], BF, tag="th")
            nc.vector.tensor_tensor(th[:], xhat[:], sgsb[:, b, 2 * CA:3 * CA], OP.mult)
            nc.vector.tensor_tensor(th[:], th[:], sgsb[:, b, 3 * CA:4 * CA], OP.add)
            ahT = pool.tile([128, KT_A, 128], BF, tag="ahT")
            thT = pool.tile([128, KT_A, 128], BF, tag="thT")
            for src, dst in ((ah, ahT), (th, thT)):
                pt = psp.tile([128, JT, 128], BF, tag="t8")
                for kt in range(KT_A):
                    nc.tensor.transpose(pt[:, kt, :], src[:, bass.ts(kt, 128)],
                                        ident[:])
                if src is ah:
                    nc.scalar.copy(out=dst[:], in_=pt[:, 0:KT_A, :])
                else:
                    nc.vector.tensor_copy(dst[:], pt[:, 0:KT_A, :])
            return ahT, thT

        def kv_pack_gather(b, ahT, pool, psp):
            """k/v for own rows; head-padded kT halves + v rows gathered."""
            wkv_sb = wpool.tile([128, KT_A, 2 * CA], BF, tag="wA")
            nc.sync.dma_start(
                out=wkv_sb,
                in_=wkv_in[:].rearrange("(kt p) b m -> p kt b m", p=128)[:, :, b, :])
            k_own = pool.tile([R, CA], BF, tag="k_own")
            v_own = pool.tile([R, CA], BF, tag="v_own")
            for m in range(3):  # 1536/512
                ps = psp.tile([R, 512], F32, tag="ps")
                for kt in range(KT_A):
                    nc.tensor.matmul(ps[:], ahT[:, kt, :],
                                     wkv_sb[:, kt, bass.ts(m, 512)],
                                     start=(kt == 0), stop=(kt == KT_A - 1))
                if m == 0:
                    nc.scalar.copy(out=k_own[:, 0:512], in_=ps[:])
                elif m == 1:
                    nc.scalar.copy(out=k_own[:, 512:768], in_=ps[:, 0:256])
                    nc.scalar.copy(out=v_own[:, 0:256], in_=ps[:, 256:512])
                else:
                    nc.scalar.copy(out=v_own[:, 256:768], in_=ps[:])
            # per-head transposes into 64-aligned psum slots (pad 48->64)
            pt = psp.tile([128, JT, 128], BF, tag="t8")
            for h in range(H):
                nc.tensor.transpose(pt[64 * (h % 2):64 * (h % 2) + D, h // 2, :],
                                    k_own[:, h * D:(h + 1) * D], ident[:])
            kT_e = pool.tile([D, 8, 128], BF, tag="kT_e")
            kT_o = pool.tile([D, 8, 128], BF, tag="kT_o")
            nc.vector.tensor_copy(kT_e[:], pt[0:D, :, :])
            nc.vector.tensor_copy(kT_o[:], pt[64:64 + D, :, :])
            kv_inb = dramc.tile([KV_TOT], BF, tag="kvin")
            kv_outb = dramc.tile([NCORE * KV_TOT], BF, tag="kvout",
                                 addr_space="Shared")
            KH = KV_K // 2
            nc.gpsimd.dma_start(
                out=kv_inb[0:KH].rearrange("(d x) -> d x", d=D),
                in_=kT_e[:].rearrange("d t j -> d (t j)"))
            nc.gpsimd.dma_start(
                out=kv_inb[KH:KV_K].rearrange("(d x) -> d x", d=D),
                in_=kT_o[:].rearrange("d t j -> d (t j)"))
            nc.gpsimd.dma_start(
                out=kv_inb[KV_K:].rearrange("(j c) -> j c", j=R), in_=v_own[:])
            nc.gpsimd.collective_compute(
                "AllGather", OP.bypass,
                replica_groups=[list(range(NCORE))],
                ins=[kv_inb[:].opt()], outs=[kv_outb[:].opt()])
            return kv_outb

        def kv_unpack(kv_outb):
            kv_outr = kv_outb[:].rearrange("(r x) -> r x", r=NCORE)
            KH = KV_K // 2
            kT_f = kvg.tile([64 + D, 8, NCORE, 128], BF, tag="kTf")
            nc.gpsimd.dma_start(
                out=kT_f[0:D, :, :, :],
                in_=kv_outr[:, 0:KH].rearrange("r (d t j) -> d t r j", d=D, t=8))
            nc.gpsimd.dma_start(
                out=kT_f[64:64 + D, :, :, :],
                in_=kv_outr[:, KH:KV_K].rearrange(
                    "r (d t j) -> d t r j", d=D, t=8))
            kT_fe = kT_f[0:D, :, :, :]
            kT_fo = kT_f[64:64 + D, :, :, :]
            v_full = kvg.tile([128, NCORE, CA], BF, tag="vf")
            nc.gpsimd.dma_start(
                out=v_full,
                in_=kv_outr[:, KV_K:].rearrange("r (j c) -> j r c", j=R))
            return (kT_fe, kT_fo), v_full

        def qg_own(b, ahT, pool, psp):
            """q (row + bias, then transposed to even/odd head tiles), gate g."""
            wqg_sb = wpool.tile([128, KT_A, 2 * CA], BF, tag="wB")
            nc.sync.dma_start(
                out=wqg_sb,
                in_=wqg_in[:].rearrange("(kt p) b m -> p kt b m", p=128)[:, :, b, :])
            bq_row = pool.tile([1, CA], BF, tag="bq_row")
            nc.sync.dma_start(out=bq_row, in_=bq_in[b:b + 1, :])
            bq_mat = pool.tile([R, CA], BF, tag="bq_mat")
            nc.gpsimd.partition_broadcast(bq_mat[:], bq_row[:])
            q_own = pool.tile([R, CA], BF, tag="k_own", name="q_own")
            g_own = pool.tile([R, CA], BF, tag="g_own")
            for m in range(3):
                ps = psp.tile([R, 512], F32, tag="ps")
                for kt in range(KT_A):
                    nc.tensor.matmul(ps[:], ahT[:, kt, :],
                                     wqg_sb[:, kt, bass.ts(m, 512)],
                                     start=(kt == 0), stop=(kt == KT_A - 1))
                if m == 0:
                    nc.vector.tensor_tensor(q_own[:, 0:512], ps[:],
                                            bq_mat[:, 0:512], OP.add)
                elif m == 1:
                    nc.vector.tensor_tensor(q_own[:, 512:768], ps[:, 0:256],
                                            bq_mat[:, 512:768], OP.add)
                    nc.scalar.activation(out=g_own[:, 0:256], in_=ps[:, 256:512],
                                         func=ACTF.Sigmoid, bias=0.0, scale=1.0)
                else:
                    nc.scalar.activation(out=g_own[:, 256:768], in_=ps[:],
                                         func=ACTF.Sigmoid, bias=0.0, scale=1.0)
            pt = psp.tile([128, JT, 128], BF, tag="t8")
            for h in range(H):
                nc.tensor.transpose(pt[64 * (h % 2):64 * (h % 2) + D, h // 2, :],
                                    q_own[:, h * D:(h + 1) * D], ident[:])
            qT_f = pool.tile([64 + D, 8, 128], BF, tag="qT_f")
            nc.scalar.copy(out=qT_f[0:D, :, :], in_=pt[0:D, :, :])
            nc.scalar.copy(out=qT_f[64:64 + D, :, :], in_=pt[64:64 + D, :, :])
            return (qT_f[0:D, :, :], qT_f[64:64 + D, :, :]), g_own

        def transition(b, thT, pool, psp):
            """hidT = silu(th@wsw)^T * (th@wg2)^T in [hid, i] tiles; then
            tr = gate_tr * (hidden @ w_out) accumulated later."""
            wsw_sb = wpool.tile([128, KT_A, NHID], BF, tag="wA", name="wsw_sb")
            nc.sync.dma_start(
                out=wsw_sb,
                in_=wsw_in[:].rearrange("(kt p) b m -> p kt b m", p=128)[:, :, b, :])
            wg2_sb = wpool.tile([128, KT_A, NHID], BF, tag="wC")
            nc.sync.dma_start(
                out=wg2_sb,
                in_=wg2_in[:].rearrange("(kt p) b m -> p kt b m", p=128)[:, :, b, :])
            # hidden row-form [R, NHID] via 3 psum chunks; then transpose.
            sw_row = pool.tile([R, NHID], BF, tag="sw_row")
            for m in range(3):
                ps = psp.tile([R, 512], F32, tag="ps")
                for kt in range(KT_A):
                    nc.tensor.matmul(ps[:], thT[:, kt, :],
                                     wsw_sb[:, kt, bass.ts(m, 512)],
                                     start=(kt == 0), stop=(kt == KT_A - 1))
                nc.scalar.activation(out=sw_row[:, bass.ts(m, 512)], in_=ps[:],
                                     func=ACTF.Silu, bias=0.0, scale=1.0)
            hid_row = pool.tile([R, NHID], BF, tag="hid_row")
            for m in range(3):
                ps = psp.tile([R, 512], F32, tag="ps")
                for kt in range(KT_A):
                    nc.tensor.matmul(ps[:], thT[:, kt, :],
                                     wg2_sb[:, kt, bass.ts(m, 512)],
                                     start=(kt == 0), stop=(kt == KT_A - 1))
                g2c = pool.tile([R, 512], BF, tag="g2c")
                nc.scalar.copy(out=g2c, in_=ps[:])
                nc.vector.tensor_tensor(hid_row[:, bass.ts(m, 512)],
                                        sw_row[:, bass.ts(m, 512)], g2c[:],
                                        OP.mult)
            hidT = pool.tile([128, KT_H, 128], BF, tag="hidT")
            for g_ in range(2):
                pt = psp.tile([128, JT, 128], BF, tag="t8")
                for q in range(6):
                    kt = g_ * 6 + q
                    nc.tensor.transpose(pt[:, q, :], hid_row[:, bass.ts(kt, 128)],
                                        ident[:])
                if g_ % 2 == 0:
                    nc.scalar.copy(out=hidT[:, 0:6, :], in_=pt[:, 0:6, :])
                else:
                    nc.vector.tensor_copy(hidT[:, 6:12, :], pt[:, 0:6, :])
            return hidT

        def attention(b, qT, g_own, kT_full, v_full, pool, psp):
            """Row-softmax attention with unnormalized exp; 1/sum folded into
            the g*o stage. Returns go_row [R, CA] bf16."""
            qT_e, qT_o = qT
            kT_fe, kT_fo = kT_full
            sums = pool.tile([R, H, 2], F32, tag="sums")
            go_row = pool.tile([R, CA], BF, tag="v_own", name="go_row")
            for h in range(H):
                qT_h = (qT_e if h % 2 == 0 else qT_o)[:, h // 2, :]
                kT_h = (kT_fe if h % 2 == 0 else kT_fo)[:, h // 2, :, :]
                attn = att.tile([R, S], BF, tag="attn")
                zb_t = att.tile([R, S], BF, tag="zbt")
                nc.gpsimd.dma_start(out=zb_t, in_=zbeta_dr[b * H + h, :, :])
                bh_ = 1 + b * H + h
                ps_s = psp.tile([R, S], F32, tag="ps2")
                for jc in range(2):
                    nc.tensor.matmul(
                        ps_s[:, bass.ts(jc, 512)], qT_h,
                        kT_h.rearrange("d r j -> d (r j)")[:, bass.ts(jc, 512)],
                        start=True, stop=True)
                nc.vector.tensor_tensor(ps_s[:], ps_s[:], zb_t[:], OP.add)
                nc.scalar.activation(out=attn, in_=ps_s[:], func=ACTF.Exp,
                                     bias=biases[:, bh_:bh_ + 1], scale=1.0,
                                     accum_out=sums[:, h, 0:1])
                attnT = att.tile([128, JT, 128], BF, tag="attnT")
                pt = psp.tile([128, JT, 128], BF, tag="t8")
                for jt in range(JT):
                    nc.tensor.transpose(pt[:, jt, :],
                                        attn[:, bass.ts(jt, 128)], ident[:])
                if h % 2 == 0:
                    nc.vector.tensor_copy(attnT[:], pt[:])
                else:
                    nc.scalar.copy(out=attnT[:], in_=pt[:])
                # o[i, d] for this head: lhsT = attnT tiles, rhs = v cols
                ps_o = psp.tile([R, D], F32, tag="ps")
                for jt in range(JT):
                    nc.tensor.matmul(ps_o[:], attnT[:, jt, :],
                                     v_full[:, jt, h * D:(h + 1) * D],
                                     start=(jt == 0), stop=(jt == JT - 1))
                rec = att.tile([R, 1], F32, tag="rec")
                nc.vector.reciprocal(out=rec, in_=sums[:, h, 0:1])
                tmp = att.tile([R, D], F32, tag="tmp")
                nc.vector.tensor_scalar(tmp[:], ps_o[:], rec[:, 0:1], None,
                                        OP.mult)
                nc.vector.tensor_tensor(go_row[:, h * D:(h + 1) * D], tmp[:],
                                        g_own[:, h * D:(h + 1) * D], OP.mult)
            return go_row

        def tr_out(b, hidT, pool, psp):
            """tr = gate_t*(hidden@w_out); runs during the gather/attention."""
            wout_sb = wpool.tile([128, KT_H, CA], BF, tag="wC",
                                 padded_shape=[128, KT_H, CA])
            nc.sync.dma_start(
                out=wout_sb,
                in_=wout_in[:].rearrange("(kt p) b m -> p kt b m", p=128)[:, :, b, :])
            tr = pool.tile([R, CA], F32, tag="tr")
            for m in range(2):
                n0, n1 = (0, 512) if m == 0 else (512, 768)
                ps2 = psp.tile([R, 512], F32, tag="ps")
                for kt in range(KT_H):
                    nc.tensor.matmul(ps2[:, 0:n1 - n0], hidT[:, kt, :],
                                     wout_sb[:, kt, n0:n1],
                                     start=(kt == 0), stop=(kt == KT_H - 1))
                nc.vector.tensor_tensor(tr[:, n0:n1], ps2[:, 0:n1 - n0],
                                        gts[:, b, CA + n0:CA + n1], OP.mult)
            return tr

        def out_proj_and_update(b, go_row, tr, pool, psp):
            """a = gate_a*(g*o)@wo + tr, written to a_sb."""
            wo_sb = wpool.tile([128, KT_A, CA], BF, tag="wB",
                               padded_shape=[128, KT_A, 2 * CA])
            nc.sync.dma_start(
                out=wo_sb,
                in_=wo_in[:].rearrange("(kt p) b m -> p kt b m", p=128)[:, :, b, :])
            goT = pool.tile([128, KT_A, 128], BF, tag="goT")
            pt = psp.tile([128, JT, 128], BF, tag="t8")
            for kt in range(KT_A):
                nc.tensor.transpose(pt[:, kt, :], go_row[:, bass.ts(kt, 128)],
                                    ident[:])
            nc.scalar.copy(out=goT[:], in_=pt[:, 0:KT_A, :])
            for m in range(2):
                n0, n1 = (0, 512) if m == 0 else (512, 768)
                ps = psp.tile([R, 512], F32, tag="ps")
                for kt in range(KT_A):
                    nc.tensor.matmul(ps[:, 0:n1 - n0], goT[:, kt, :],
                                     wo_sb[:, kt, n0:n1],
                                     start=(kt == 0), stop=(kt == KT_A - 1))
                batt = pool.tile([R, 512], F32, tag="batt")
                nc.vector.tensor_tensor(batt[:, 0:n1 - n0], ps[:, 0:n1 - n0],
                                        gts[:, b, n0:n1], OP.mult)
                nc.vector.tensor_tensor(a_sb[:, n0:n1], batt[:, 0:n1 - n0],
                                        tr[:, n0:n1], OP.add)

        # =========== block 0 front (kv gather launches before z-prep) ===========
        psA = ctx.enter_context(tc.tile_pool(name="psA", bufs=2, space="PSUM"))
        ahT0, thT0 = ada_ln_own(0, blk, psA)
        kv_out0 = kv_pack_gather(0, ahT0, blk, psA)

        # =========== z preprocessing (once), groups of 4 i-rows ===========
        # software-pipelined: group g+1's loads+stats are emitted before
        # group g's normalize so the DVE bn_stats stream never stalls.
        with tc.tile_pool(name="zslab", bufs=3) as zsl, \
             tc.tile_pool(name="zsm", bufs=3) as zsm:
            NG = R // 4
            zg = {}

            def emit_load_stats(g):
                zts = []
                st8 = zsm.tile([128, 4, JT, 6], F32, tag="st", bufs=3,
                               name="st8")
                for di in range(4):
                    zt = zsl.tile([128, JT, CZ], BF, tag="z", bufs=7, name="zt")
                    nc.sync.dma_start(out=zt[:].rearrange("p jt c -> p (jt c)"),
                                      in_=z_in[g * 4 + di])
                    zts.append(zt)
                    for jt in range(JT):
                        nc.vector.bn_stats(out=st8[:, di, jt, :], in_=zt[:, jt, :])
                zg[g] = (zts, st8)

            emit_load_stats(0)
            for i4 in range(NG):
                zts, st8 = zg.pop(i4)
                # pooled even/odd moments, batched over 4 i-rows:
                # mean = (m_e+m_o)/2; var = (M2_e+M2_o)/CZ + ((m_e-m_o)/2)^2
                J4 = 4 * JT
                s1 = st8[:].rearrange("p di jt s -> p (di jt) s")
                mrow = zsm.tile([128, 4, JT], F32, tag="mrow", bufs=2)
                m2 = mrow[:].rearrange("p di jt -> p (di jt)")
                nc.vector.tensor_tensor(m2, s1[:, :, 1], s1[:, :, 4], OP.add)
                nc.vector.tensor_scalar(m2, m2, 0.5, None, OP.mult)
                dm = zsm.tile([128, J4], F32, tag="dm", bufs=2)
                nc.vector.tensor_tensor(dm[:], s1[:, :, 1], s1[:, :, 4],
                                        OP.subtract)
                nc.vector.tensor_tensor(dm[:], dm[:], dm[:], OP.mult)
                nc.vector.tensor_scalar(dm[:], dm[:], 0.25, None, OP.mult)
                var = zsm.tile([128, J4], F32, tag="var", bufs=2)
                nc.vector.tensor_tensor(var[:], s1[:, :, 2], s1[:, :, 5], OP.add)
                nc.vector.tensor_scalar(var[:], var[:], 1.0 / CZ, None, OP.mult)
                nc.vector.tensor_tensor(var[:], var[:], dm[:], OP.add)
                rst = zsm.tile([128, 4, JT], F32, tag="rst", bufs=2)
                r2 = rst[:].rearrange("p di jt -> p (di jt)")
                nc.scalar.activation(out=r2, in_=var[:], func=ACTF.Sqrt,
                                     bias=biases[:, 0:1], scale=1.0)
                nc.vector.reciprocal(out=r2, in_=r2)
                if i4 + 1 < NG:
                    emit_load_stats(i4 + 1)
                for dp in range(2):  # pairs of i-rows share one zb psum tile
                    zb = psA.tile([128, S], F32, tag="ps2")
                    bsel2 = zsl.tile([H, 2, S], BF, tag="bsel", bufs=1)
                    i0p = i4 * 4 + dp * 2
                    nc.sync.dma_start(out=bsel2, in_=betaT_in[:, i0p:i0p + 2, :])
                    for dj in range(2):
                        di = dp * 2 + dj
                        i = i4 * 4 + di
                        zt = zts[di]
                        bsel = bsel2[:, dj, :]
                        zh = zsm.tile([128, JT, CZ], BF, tag="zh", bufs=2)
                        eng = nc.vector if di == 0 else nc.gpsimd
                        for jt in range(JT):
                            eng.tensor_scalar(zh[:, jt, :], zt[:, jt, :],
                                              mrow[:, di, jt:jt + 1],
                                              rst[:, di, jt:jt + 1],
                                              OP.subtract, OP.mult)
                        zhT = zsm.tile([128, JT, 128], BF, tag="zhT", bufs=2)
                        pt = psA.tile([128, JT, 128], BF, tag="t8")
                        for jt in range(JT):
                            nc.tensor.transpose(pt[:, jt, :], zh[:, jt, :],
                                                ident[:])
                        nc.scalar.copy(out=zhT[:], in_=pt[:])
                        zrow = zb[64 * dj:64 * dj + 64, :]
                        for jc in range(2):
                            nc.tensor.matmul(zrow[:, bass.ts(jc, 512)], wz_sb[:],
                                             zhT[:].rearrange("p jt j -> p (jt j)")[:, bass.ts(jc, 512)],
                                             start=True, stop=False)
                            nc.tensor.matmul(zrow[:, bass.ts(jc, 512)], sel_sb[:],
                                             bsel[:, bass.ts(jc, 512)],
                                             start=False, stop=True)
                    zbs = zsm.tile([128, S], BF, tag="zbs", bufs=2)
                    nc.scalar.copy(out=zbs, in_=zb[:])
                    i0 = i4 * 4 + dp * 2
                    nc.sync.dma_start(out=zbeta_dr[:, i0, :], in_=zbs[0:64, :])
                    nc.sync.dma_start(out=zbeta_dr[:, i0 + 1, :],
                                      in_=zbs[64:128, :])

        # =========== block loop ===========
        qT0, g0 = qg_own(0, ahT0, blk, psA)
        hidT0 = transition(0, thT0, blk, psA)
        for b in range(NB):
            if b == 0:
                kv_outb, qT, g_own, hidT = kv_out0, qT0, g0, hidT0
            else:
                ahT, thT = ada_ln_own(b, blk, psA)
                kv_outb = kv_pack_gather(b, ahT, blk, psA)
                qT, g_own = qg_own(b, ahT, blk, psA)
                hidT = transition(b, thT, blk, psA)
            kT_full, v_full = kv_unpack(kv_outb)
            tr = tr_out(b, hidT, blk, psA)
            go_row = attention(b, qT, g_own, kT_full, v_full, blk, psA)
            out_proj_and_update(b, go_row, tr, blk, psA)

        nc.sync.dma_start(out=a_out[:], in_=a_sb[:])

    nc.finalize()
    return nc


def _prep_inputs(a, s, z, beta, ln_s_w_attn, wg_attn, wb_attn, wq, bq, wk, wv,
                 ln_z_w, ln_z_b, wpb, wgate, wo, wsg_attn, bsg_attn,
                 ln_s_w_tr, wg_tr, wb_tr, w_swish, w_gate2, wsg_tr, bsg_tr, w_out):
    bf = ml_dtypes.bfloat16
    f32 = np.float32
    scale = 1.0 / np.sqrt(np.float32(D))

    # folded weights (shared across cores)
    wz = np.concatenate([ln_z_w[i][:, None] * wpb[i] for i in range(NB)],
                        axis=1).astype(bf)                       # [CZ, NB*H]
    bias_pb = np.concatenate([ln_z_b[i] @ wpb[i] for i in range(NB)])  # [NB*H]
    sel = np.tile(np.eye(H, dtype=np.float32), (1, NB)).astype(bf)  # [H, NB*H]
    wsn = np.stack([np.concatenate(
        [ln_s_w_attn[i][:, None] * wg_attn[i], ln_s_w_attn[i][:, None] * wb_attn[i],
         ln_s_w_tr[i][:, None] * wg_tr[i], ln_s_w_tr[i][:, None] * wb_tr[i]],
        axis=1) for i in range(NB)], axis=1).astype(bf)          # [CS, NB, 4CA]
    wsr = np.stack([np.concatenate(
        [np.concatenate([wsg_attn[i], bsg_attn[i][None, :]], 0),
         np.concatenate([wsg_tr[i], bsg_tr[i][None, :]], 0)], axis=1)
        for i in range(NB)], axis=1).astype(bf)                  # [CS+1, NB, 2CA]
    wkv = np.stack([np.concatenate([wk[i], wv[i]], 1) for i in range(NB)],
                   axis=1).astype(bf)                            # [CA, NB, 2CA]
    wqg = np.stack([np.concatenate([wq[i] * scale, wgate[i]], 1)
                    for i in range(NB)], axis=1).astype(bf)
    bqe = (bq * scale).astype(bf)                               # [NB, CA]
    wsw = np.stack([w_swish[i] for i in range(NB)], axis=1).astype(bf)
    wg2 = np.stack([w_gate2[i] for i in range(NB)], axis=1).astype(bf)
    wob = np.stack([wo[i] for i in range(NB)], axis=1).astype(bf)
    wout = np.stack([w_out[i] for i in range(NB)], axis=1).astype(bf)

    shared = dict(wz_in=np.ascontiguousarray(wz),
                  sel_in=np.ascontiguousarray(sel),
                  wsn_in=np.ascontiguousarray(wsn),
                  wsr_in=np.ascontiguousarray(wsr),
                  wkv_in=np.ascontiguousarray(wkv),
                  wqg_in=np.ascontiguousarray(wqg),
                  bq_in=np.ascontiguousarray(bqe),
                  wsw_in=np.ascontiguousarray(wsw),
                  wg2_in=np.ascontiguousarray(wg2),
                  wo_in=np.ascontiguousarray(wob),
                  wout_in=np.ascontiguousarray(wout))

    a2 = a.reshape(S, CA).astype(f32)
    s2 = s.reshape(S, CS).astype(f32)
    z2 = z.reshape(S, S, CZ).astype(bf)
    betaT = np.ascontiguousarray(
        beta.reshape(S, S, H).transpose(2, 0, 1)).astype(bf)     # [H, S, S]

    in_maps = []
    for c in range(NCORE):
        rows = slice(c * R, (c + 1) * R)
        m = dict(shared)
        m["a_in"] = np.ascontiguousarray(a2[rows])
        m["s_in"] = np.ascontiguousarray(s2[rows])
        # [i, jp, jt, c] so SBUF partition lines are (jt, c) = 2KB contiguous
        zj = z2[rows].reshape(R, JT, 128, CZ).transpose(0, 2, 1, 3)
        m["z_in"] = np.ascontiguousarray(zj.reshape(R, 128, JT * CZ))
        m["betaT_in"] = np.ascontiguousarray(betaT[:, rows, :])
        in_maps.append(m)
    return in_maps, [float(x) for x in bias_pb]


_CACHE = {}


def kernel(**inputs):
    inputs = {k: np.asarray(v) for k, v in inputs.items()}
    in_maps, bias_pb = _prep_inputs(**inputs)
    key = tuple(bias_pb)
    if key not in _CACHE:
        _CACHE.clear()
        _CACHE[key] = build_program(bias_pb)
    nc = _CACHE[key]
    res = run_bass_kernel_spmd(nc, in_maps, core_ids=list(range(NCORE)),
                               trace=False)
    out = np.concatenate([res.results[c]["a_out"] for c in range(NCORE)], axis=0)
    return out.reshape(1, S, CA).astype(np.float32)


if __name__ == "__main__":
    import reference
    ins = {k: np.asarray(v) for k, v in reference.setup_inputs().items()}
    exp = np.asarray(reference.reference(**ins))
    act = kernel(**ins)
    err = np.abs(act - exp).max() / (np.abs(exp).max() + 1e-9)
    print("rel err:", err)
